# revision 3
# baseline (speedup 1.0000x reference)
"""AttnBlockpp (GroupNorm -> q/k/v NIN -> full spatial attention -> NIN ->
residual) for Trainium2, data-parallel over batch across 8 NeuronCores.
Per-core shard: 2 samples of [512, 32, 32] (N = 1024 spatial tokens).

Host-side pre/post-processing (extends the weight folding the original
baseline shipped with):

    M01 = W0 @ W1^T             scores[m,n] = hn_m^T M01^T hn_n + r1.hn_m
    W23 = W2 @ W3               o[n,:] = attn-avg over keys m of (hn^T W23)[m,:]
    b23 = W3^T b2 + b3          r1  = W1 @ b0
    hn  = groupnorm(x)          (exact f32 stats, as the reference)
    epilogue: out = x + b23 + num/den   (softmax normalizer + residual)

The query-side b1 term and the b0.b1 constant cancel inside the softmax
over keys m; the key-side term r1.hn_m rides the exp() bias together with
a softmax-invariant shift C0 that keeps exp() inside the fp8 range.

All four large matmuls run as fp8 DoubleRow (two 128-deep k-tiles per
instruction at 0.5 PE cycles/row = 4x the fp32r/bf16 rate).  Plain e4m3
operands are too noisy for the 2e-2 gate, so every operand is carried as
a TWO-TERM e4m3 pair T = hi + lo (lo = fp8(T - hi), ~0.13% effective
error) and each product keeps three cross terms (hi*hi, lo*hi, hi*lo).
eS = exp(scores - C0) is stored once in e5m2 - its 22-nat range covers
the heavy-tailed scores where e4m3's 11.7-nat window cannot, and its 7%
weight noise is self-cancelling for peaked softmax rows because num and
den use the same quantized eS.

Per sample on the device (PSUM f32 accumulation; WS=16 pre-scale on
M01/W23 puts their entries in the e4m3 normal range and cancels between
the exp scale, the 16.0-valued ones vector of den, and num/den):

    g   [d,n] = (16 M01)^T hn    48 DR matmuls -> ACT hi / DVE lo -> g8 pair
    vW  [m,d] = hn^T (16 W23)    48 DR matmuls -> ACT hi / DVE lo -> vW8 pair
    St  [m,n] = hn^T g8          96 DR matmuls
    eS  [m,n] = exp(St*s + rho[m])   ACT Exp -> e5m2
    den [n]   = 16 sum_m eS      32 free DR matmuls vs ones16
    num [n,d] = eS^T (vW8 hi+lo) 64 DR matmuls -> evac -> DMA (f32)

Schedule notes (v2, tuned against the TimelineSim cost model):
  * One tiny matmul over a gpsimd-memset scratch tile fires at t~250 to
    anchor the PE p-state ramp (the ramp window survives PE idle, so the
    old 8-matmul warm-up burn is replaced by real work at full clock
    from ~2.4us).
  * All input tensors are staged p-major in DRAM (one descriptor per
    partition), making the gpsimd SWDGE lane ~1us per tensor; the first
    sample's hn rides the HWDGE lane in arrival-ordered chunks and the
    front fill consumes them wave-by-wave (term-major across a quad of
    PSUM tiles).
  * Tail: the last sample's evacuations alternate ACT/DVE, output DMAs
    spread across the sync/scalar/gpsimd queues, and the final tile is
    split in half so the end-of-kernel evac+DMA+semaphore drain chain is
    short.
"""

import numpy as np
import ml_dtypes

import concourse.bass as bass
import concourse.mybir as mybir
import concourse.tile as tile
from concourse import bacc
from concourse.bass_utils import run_bass_kernel_spmd

NCORES = 8
B_FULL, C, H, W = 16, 512, 32, 32
B_LOC = B_FULL // NCORES          # samples per core
N = H * W                         # spatial tokens
G = 32                            # groupnorm groups
EPS = 1e-6
P = 128
NKO = C // P                      # channel chunks (4)
NMM = N // P                      # spatial chunks (8)
NH = 512                          # n-half size
WS = 16.0                         # fp8 pre-scale on M01/W23
C0 = 6.0                          # softmax-invariant exp shift
SEXP = float(C) ** -0.5 / WS      # St psum carries one factor of WS (M01)

F32 = mybir.dt.float32
F8 = mybir.dt.float8e4
F8E5 = mybir.dt.float8e5
E4M3 = ml_dtypes.float8_e4m3
Act = mybir.ActivationFunctionType
DR = mybir.MatmulPerfMode.DoubleRow


def _build(fast_bias):
    nc = bacc.Bacc("TRN2", target_bir_lowering=False, debug=False)

    # p-major DRAM layouts: one contiguous run per partition per chunk.
    hn_d = nc.dram_tensor("hn8", [B_LOC, 2, P, NKO, N], F8,
                          kind="ExternalInput").ap()
    m01_d = nc.dram_tensor("M01", [2, P, NKO, C], F8,
                           kind="ExternalInput").ap()
    w23_d = nc.dram_tensor("W23", [2, P, NKO, C], F8,
                           kind="ExternalInput").ap()
    rho_d = nc.dram_tensor("rho", [B_LOC, P, NMM], F32,
                           kind="ExternalInput").ap()
    num_d = nc.dram_tensor("num", [B_LOC, NMM, P, C], F32,
                           kind="ExternalOutput").ap()
    den_d = nc.dram_tensor("den", [B_LOC, P, NMM], F32,
                           kind="ExternalOutput").ap()

    ones_np = np.full((P, 2, 1), WS, dtype=E4M3)
    ones_d = nc.inline_tensor(ones_np, name="ones16").ap()

    with tile.TileContext(nc) as tc:
        _body(tc, hn_d, m01_d, w23_d, rho_d, ones_d, num_d, den_d, fast_bias)
    nc.compile()
    return nc


def _body(tc, hn_d, m01_d, w23_d, rho_d, ones_d, num_d, den_d, fast_bias):
    nc = tc.nc
    import contextlib

    with contextlib.ExitStack() as ctx:
        singles = ctx.enter_context(tc.tile_pool(name="singles", bufs=1))
        hnpool = ctx.enter_context(tc.tile_pool(name="hnpool", bufs=2))
        gpool = ctx.enter_context(tc.tile_pool(name="gpool", bufs=2))
        vpool = ctx.enter_context(tc.tile_pool(name="vpool", bufs=2))
        espool = ctx.enter_context(tc.tile_pool(name="espool", bufs=2))
        ypool = ctx.enter_context(tc.tile_pool(name="ypool", bufs=4))
        ps = ctx.enter_context(tc.tile_pool(name="ps", bufs=7, space="PSUM"))
        psd = ctx.enter_context(tc.tile_pool(name="psd", bufs=1, space="PSUM"))

        hn_sb = []
        rho_sb = []

        g8 = [None] * B_LOC
        vW8 = [None] * B_LOC
        eS8 = [None] * B_LOC
        den_t = psd.tile([P, B_LOC, NMM], F32, name="den_t", tag="den",
                         space="PSUM")

        # p-state ramp anchor: one tiny DR matmul over a gpsimd-memset
        # scratch tile.  The ramp window is keyed to the FIRST PE matmul
        # and survives idle, so by the time the input DMAs land (~2.4us)
        # the clock is most of the way to full speed.  The garbage result
        # lands in den_t partitions/columns that every den matmul later
        # resets with start=True.
        scr = singles.tile([P, 2, 32], F8, name="scr", tag="scr")
        nc.gpsimd.memset(scr, 0)
        nc.tensor.matmul(den_t[0:32, :, :], scr[:, :, 0:32], scr[:, :, 0:16],
                         start=True, stop=True, perf_mode=DR)

        # ---- input DMAs, priority-ordered ----
        # sync/SP HWDGE lane (byte-rate bound): sample-0 hn in arrival-
        # ordered chunks, then the w23 pair.
        t0 = hnpool.tile([P, 2, NKO, N], F8, name="hn_s0", tag="hn")
        hn_sb.append(t0)
        nc.sync.dma_start(t0[:, 0, 0:2, 0:NH], hn_d[0, 0, :, 0:2, 0:NH])
        nc.sync.dma_start(t0[:, 0, 0:2, NH:N], hn_d[0, 0, :, 0:2, NH:N])
        nc.sync.dma_start(t0[:, 0, 2:4, 0:NH], hn_d[0, 0, :, 2:4, 0:NH])
        nc.sync.dma_start(t0[:, 0, 2:4, NH:N], hn_d[0, 0, :, 2:4, NH:N])
        nc.sync.dma_start(t0[:, 1, 0:2, :], hn_d[0, 1, :, 0:2, :])
        nc.sync.dma_start(t0[:, 1, 2:4, 0:NH], hn_d[0, 1, :, 2:4, 0:NH])
        nc.sync.dma_start(t0[:, 1, 2:4, NH:N], hn_d[0, 1, :, 2:4, NH:N])
        w23_sb = singles.tile([P, 2, NKO, C], F8, name="w23_sb", tag="w23")
        nc.sync.dma_start(w23_sb[:, 0], w23_d[0])
        nc.sync.dma_start(w23_sb[:, 1], w23_d[1])

        # gpsimd/Pool SWDGE lane (descriptor bound; p-major = 1 desc per
        # partition): the m01 pair first (gates the very first matmul
        # together with the first hn chunk), then sample-1 hn.
        m01_sb = singles.tile([P, 2, NKO, C], F8, name="m01_sb", tag="m01")
        nc.gpsimd.dma_start(m01_sb[:, 0], m01_d[0])
        nc.gpsimd.dma_start(m01_sb[:, 1], m01_d[1])
        t1 = hnpool.tile([P, 2, NKO, N], F8, name="hn_s1", tag="hn")
        hn_sb.append(t1)
        nc.gpsimd.dma_start(t1[:, 0], hn_d[1, 0])
        nc.gpsimd.dma_start(t1[:, 1], hn_d[1, 1])

        # scalar/ACT HWDGE lane: the small stuff.
        ones_sb = singles.tile([P, 2, 1], F8, name="ones_sb", tag="ones")
        nc.scalar.dma_start(ones_sb, ones_d)
        r0 = singles.tile([P, NMM], F32, name="rho_s0", tag="rho0")
        nc.scalar.dma_start(r0, rho_d[0])
        rho_sb.append(r0)
        r1 = singles.tile([P, NMM], F32, name="rho_s1", tag="rho1")
        nc.scalar.dma_start(r1, rho_d[1])
        rho_sb.append(r1)

        # two-term operand pairs (hi*hi, lo*hi, hi*lo; lo*lo dropped at
        # ~0.13% magnitude).
        TERMS = ((0, 0), (1, 0), (0, 1))

        def evac2(dst_hi, dst_lo, pt):
            """PSUM -> two-term fp8: hi on ACT, lo (residual) on DVE."""
            nc.scalar.activation(dst_hi, pt, Act.Identity)
            nc.vector.tensor_tensor(dst_lo, pt, dst_hi,
                                    mybir.AluOpType.subtract)

        def front(s):
            """g = (16 M01)^T hn and vW = hn^T (16 W23): three two-term
            cross products accumulated in PSUM, evacuated to fp8 pairs."""
            hn = hn_sb[s]
            g8[s] = gpool.tile([P, 2, NKO, N], F8, name=f"g8_s{s}", tag="g8")
            vW8[s] = vpool.tile([P, 2, NMM, C], F8, name=f"vW8_s{s}",
                                tag="vW8")

            def g_tile(dc, nh):
                gt = ps.tile([P, NH], F32, name=f"g_{dc}_{nh}_s{s}",
                             tag="big", space="PSUM")
                k = 0
                for tm, th in TERMS:
                    for j in range(2):
                        nc.tensor.matmul(
                            gt, m01_sb[:, tm, 2 * j:2 * j + 2,
                                       dc * P:(dc + 1) * P],
                            hn[:, th, 2 * j:2 * j + 2, nh * NH:(nh + 1) * NH],
                            start=(k == 0), stop=(k == 5), perf_mode=DR)
                        k += 1
                evac2(g8[s][:, 0, dc, nh * NH:(nh + 1) * NH],
                      g8[s][:, 1, dc, nh * NH:(nh + 1) * NH], gt)

            def v_tile(mm):
                vt = ps.tile([P, NH], F32, name=f"v_{mm}_s{s}", tag="big",
                             space="PSUM")
                k = 0
                # hi*Whi, lo*Whi, hi*Wlo: w23-lo is the latest HWDGE
                # arrival, so it comes last.
                for th, tw in ((0, 0), (1, 0), (0, 1)):
                    for j in range(2):
                        nc.tensor.matmul(
                            vt, hn[:, th, 2 * j:2 * j + 2,
                                   mm * P:(mm + 1) * P],
                            w23_sb[:, tw, 2 * j:2 * j + 2, :],
                            start=(k == 0), stop=(k == 5), perf_mode=DR)
                        k += 1
                evac2(vW8[s][:, 0, mm, :], vW8[s][:, 1, mm, :], vt)

            def g_quad(quad, waves):
                """Sweep term-waves across a quad of g tiles in DMA
                arrival order: (tm, th, j) per wave, nh-minor so the
                first half of each hn chunk unblocks the wave start."""
                gts = {}
                for dc, nh in quad:
                    gts[(dc, nh)] = ps.tile([P, NH], F32,
                                            name=f"g_{dc}_{nh}_s{s}",
                                            tag="big", space="PSUM")
                nwave = len(waves)
                for wi, (tm, th, j) in enumerate(waves):
                    for dc, nh in quad:
                        nc.tensor.matmul(
                            gts[(dc, nh)],
                            m01_sb[:, tm, 2 * j:2 * j + 2,
                                   dc * P:(dc + 1) * P],
                            hn[:, th, 2 * j:2 * j + 2,
                               nh * NH:(nh + 1) * NH],
                            start=(wi == 0), stop=(wi == nwave - 1),
                            perf_mode=DR)
                for dc, nh in quad:
                    evac2(g8[s][:, 0, dc, nh * NH:(nh + 1) * NH],
                          g8[s][:, 1, dc, nh * NH:(nh + 1) * NH],
                          gts[(dc, nh)])

            if s == 0:
                # fill window: waves ordered by DMA arrival --
                # hi*hi j0 (hn-hi ko01), lo*hi j0 (m01-lo, SWDGE),
                # hi*hi j1 (hn-hi ko23), lo*hi j1, hi*lo j0 (hn-lo ko01),
                # hi*lo j1 (hn-lo ko23).  nh0 tiles lead inside each wave.
                waves = ((0, 0, 0), (1, 0, 0), (0, 0, 1),
                         (1, 0, 1), (0, 1, 0), (0, 1, 1))
                quad = [(0, 0), (1, 0), (0, 1), (1, 1)]
                g_quad(quad, waves)
                quad = [(2, 0), (3, 0), (2, 1), (3, 1)]
                g_quad(quad, waves)
                for u in range(8):
                    v_tile(u)
            else:
                # interleave g and vW tiles so the evacuation engines see
                # a steady stream instead of end-of-phase bursts
                for u in range(8):
                    g_tile(u // 2, u % 2)
                    v_tile(u)

        def scores(s, nh):
            """St = hn^T g8 (two-term both sides) for one n-half; exp ->
            eS8 (fp8 e5m2), key-side bias + overflow shift via rho."""
            hn = hn_sb[s]
            if eS8[s] is None:
                eS8[s] = espool.tile([P, NMM, N], F8E5, name=f"eS_s{s}",
                                     tag="eS")
            sl = slice(nh * NH, (nh + 1) * NH)
            for mm in range(NMM):
                st = ps.tile([P, NH], F32, name=f"st_{mm}_{nh}_s{s}",
                             tag="big", space="PSUM")
                k = 0
                for th, tg in TERMS:
                    for j in range(2):
                        nc.tensor.matmul(
                            st, hn[:, th, 2 * j:2 * j + 2,
                                   mm * P:(mm + 1) * P],
                            g8[s][:, tg, 2 * j:2 * j + 2, sl],
                            start=(k == 0), stop=(k == 5), perf_mode=DR)
                        k += 1
                nc.scalar.activation(eS8[s][:, mm, sl], st, Act.Exp,
                                     scale=SEXP, bias=rho_sb[s][:, mm:mm + 1])

        def tail(s, nh):
            """den columns (first, so den leaves early) + numerator
            matmuls (two-term vW) for one n-half; PSUM -> SBUF -> DMA."""
            eS = eS8[s]
            for nck in range(nh * 4, nh * 4 + 4):
                csl = slice(nck * P, (nck + 1) * P)
                for j in range(4):
                    nc.tensor.matmul(
                        den_t[:, s, nck:nck + 1],
                        eS[:, 2 * j:2 * j + 2, csl], ones_sb,
                        start=(j == 0), stop=(j == 3), perf_mode=DR)
            if nh == 1:
                dsb = singles.tile([P, NMM], F32, name=f"den_sb_s{s}",
                                   tag=f"densb{s}")
                nc.vector.tensor_copy(dsb, den_t[:, s, :])
                if s == 0:
                    nc.gpsimd.dma_start(den_d[s], dsb)
                else:
                    nc.scalar.dma_start(den_d[s], dsb)
            for nck in range(nh * 4, nh * 4 + 4):
                csl = slice(nck * P, (nck + 1) * P)
                nt = ps.tile([P, C], F32, name=f"n_{nck}_s{s}", tag="big",
                             space="PSUM")
                k = 0
                for tw in range(2):
                    for j in range(4):
                        nc.tensor.matmul(
                            nt, eS[:, 2 * j:2 * j + 2, csl],
                            vW8[s][:, tw, 2 * j:2 * j + 2, :],
                            start=(k == 0), stop=(k == 7), perf_mode=DR)
                        k += 1
                y = ypool.tile([P, C], F32, name=f"y_{nck}_s{s}", tag="y")
                if s == 0:
                    # sample-0 output rides the otherwise-idle SWDGE lane
                    nc.vector.tensor_copy(y, nt)
                    nc.gpsimd.dma_start(num_d[s, nck], y)
                elif nck == 7:
                    # final tile: halves evac'd on both engines and DMA'd
                    # on both HWDGE queues so the end drain is short.
                    nc.scalar.activation(y[:, 0:256], nt[:, 0:256],
                                         Act.Identity)
                    nc.scalar.dma_start(num_d[s, nck, :, 0:256], y[:, 0:256])
                    nc.vector.tensor_copy(y[:, 256:], nt[:, 256:])
                    nc.sync.dma_start(num_d[s, nck, :, 256:], y[:, 256:])
                else:
                    # alternate evac engine in the drain phase (ACT is
                    # done with exp by the second half)
                    if nh == 1 and nck % 2 == 0:
                        nc.scalar.activation(y, nt, Act.Identity)
                    else:
                        nc.vector.tensor_copy(y, nt)
                    if nck % 2 == 0:
                        nc.sync.dma_start(num_d[s, nck], y)
                    else:
                        nc.scalar.dma_start(num_d[s, nck], y)

        # software pipeline: sample-1 front/scores fill PE slack while
        # sample-0's exp (ACT) and evacuations (DVE) drain, and vice versa.
        front(0)
        scores(0, 0)
        scores(0, 1)
        front(1)
        tail(0, 0)
        scores(1, 0)
        tail(0, 1)
        scores(1, 1)
        tail(1, 0)
        tail(1, 1)


_NC_CACHE = {}


def _get_nc(fast_bias=True):
    key = bool(fast_bias)
    if key not in _NC_CACHE:
        _NC_CACHE[key] = _build(key)
    return _NC_CACHE[key]


def _groupnorm_host(x, gamma, beta):
    b, c, h, w = x.shape
    xg = x.reshape(b, G, c // G, h * w)
    mu = xg.mean(axis=(2, 3), keepdims=True)
    var = xg.var(axis=(2, 3), keepdims=True)
    xn = ((xg - mu) / np.sqrt(var + EPS)).reshape(b, c, h * w)
    return xn * gamma[None, :, None] + beta[None, :, None]


def run(inputs, trace=False):
    f64 = np.float64
    W0 = np.asarray(inputs["W0"], f64)
    W1 = np.asarray(inputs["W1"], f64)
    W2 = np.asarray(inputs["W2"], f64)
    W3 = np.asarray(inputs["W3"], f64)
    b0 = np.asarray(inputs["b0"], f64)
    b2 = np.asarray(inputs["b2"], f64)
    b3 = np.asarray(inputs["b3"], f64)

    x = np.asarray(inputs["x"], np.float32)
    gamma = np.asarray(inputs["gn_gamma"], np.float32)
    beta = np.asarray(inputs["gn_beta"], np.float32)

    hn = _groupnorm_host(x, gamma, beta)              # [B, C, N] f32
    hn_hi = hn.astype(E4M3)
    hn_lo = (hn - hn_hi.astype(np.float32)).astype(E4M3)
    hn8 = np.stack([hn_hi, hn_lo], axis=1)            # [B, 2, C, N]
    # p-major: [B, 2, P, NKO, N]
    hn8 = np.ascontiguousarray(
        hn8.reshape(B_FULL, 2, NKO, P, N).transpose(0, 1, 3, 2, 4))

    M01 = (W0 @ W1.T) * WS
    W23 = (W2 @ W3) * WS
    b23 = (W3.T @ b2 + b3).astype(np.float32)
    r1 = W1 @ b0

    fast_bias = not np.any(r1)
    s = float(C) ** -0.5
    if fast_bias:
        rho = np.full((B_FULL, N), -C0, np.float32)
    else:
        # key-side bias of q.k, shifted per sample so exp() stays in the
        # fp8 range; the shift is softmax-invariant.
        rho = s * np.einsum("c,bcn->bn", r1, hn.astype(f64))
        rho = (rho - np.maximum(rho.max(axis=1, keepdims=True), 0.0)
               - C0).astype(np.float32)
    # p-major: [B, P, NMM]
    rho_pm = np.ascontiguousarray(
        rho.reshape(B_FULL, NMM, P).transpose(0, 2, 1))

    nc = _get_nc(fast_bias)

    def two_term(a):
        a = a.astype(np.float32)
        hi = a.astype(E4M3)
        lo = (a - hi.astype(np.float32)).astype(E4M3)
        pair = np.stack([hi, lo], axis=0)             # [2, C, C]
        # p-major: [2, P, NKO, C]
        return np.ascontiguousarray(
            pair.reshape(2, NKO, P, C).transpose(0, 2, 1, 3))

    base = {
        "M01": two_term(M01),
        "W23": two_term(W23),
    }
    in_maps = []
    for cid in range(NCORES):
        sl = slice(cid * B_LOC, (cid + 1) * B_LOC)
        in_maps.append(dict(base,
                            hn8=np.ascontiguousarray(hn8[sl]),
                            rho=np.ascontiguousarray(rho_pm[sl])))
    res = run_bass_kernel_spmd(nc, in_maps, list(range(NCORES)), trace=trace)

    num = np.concatenate([r["num"] for r in res.results], axis=0)
    den = np.concatenate([r["den"] for r in res.results], axis=0)
    # num[b, nck, p, d]: n = nck*128 + p ; den[b, p, nc]: n = nc*128 + p
    num = num.reshape(B_FULL, N, C)
    den = den.transpose(0, 2, 1).reshape(B_FULL, N)
    o = num / den[:, :, None]                          # [B, N, C]
    out = x + b23[None, :, None, None] \
        + o.transpose(0, 2, 1).reshape(B_FULL, C, H, W).astype(np.float32)
    return out, res


def kernel(**inputs) -> np.ndarray:
    out, _ = run(inputs)
    return out


# revision 6
# speedup vs baseline: 1.0143x; 1.0143x over previous
"""AttnBlockpp (GroupNorm -> q/k/v NIN -> full spatial attention -> NIN ->
residual) for Trainium2, data-parallel over batch across 8 NeuronCores.
Per-core shard: 2 samples of [512, 32, 32] (N = 1024 spatial tokens).

Host-side pre/post-processing (extends the weight folding the original
baseline shipped with):

    M01 = W0 @ W1^T             scores[m,n] = hn_m^T M01^T hn_n + r1.hn_m
    W23 = W2 @ W3               o[n,:] = attn-avg over keys m of (hn^T W23)[m,:]
    b23 = W3^T b2 + b3          r1  = W1 @ b0
    hn  = groupnorm(x)          (exact f32 stats, as the reference)
    epilogue: out = x + b23 + num/den   (softmax normalizer + residual)

The query-side b1 term and the b0.b1 constant cancel inside the softmax
over keys m; the key-side term r1.hn_m rides the exp() bias together with
a softmax-invariant shift C0 that keeps exp() inside the fp8 range.

All four large matmuls run as fp8 DoubleRow (two 128-deep k-tiles per
instruction at 0.5 PE cycles/row = 4x the fp32r/bf16 rate).  Plain e4m3
operands are too noisy for the 2e-2 gate, so every operand is carried as
a TWO-TERM e4m3 pair T = hi + lo (lo = fp8(T - hi), ~0.13% effective
error) and each product keeps three cross terms (hi*hi, lo*hi, hi*lo).
eS = exp(scores - C0) is stored once in e5m2 - its 22-nat range covers
the heavy-tailed scores where e4m3's 11.7-nat window cannot, and its 7%
weight noise is self-cancelling for peaked softmax rows because num and
den use the same quantized eS.

Per sample on the device (PSUM f32 accumulation; WS=16 pre-scale on
M01/W23 puts their entries in the e4m3 normal range and cancels between
the exp scale, the 16.0-valued ones vector of den, and num/den):

    g   [d,n] = (16 M01)^T hn    48 DR matmuls -> ACT hi / DVE lo -> g8 pair
    vW  [m,d] = hn^T (16 W23)    48 DR matmuls -> ACT hi / DVE lo -> vW8 pair
    St  [m,n] = hn^T g8          96 DR matmuls
    eS  [m,n] = exp(St*s + rho[m])   ACT Exp -> e5m2
    den [n]   = 16 sum_m eS      32 free DR matmuls vs ones16
    num [n,d] = eS^T (vW8 hi+lo) 64 DR matmuls -> evac -> DMA (f32)

Schedule notes (v2, tuned against the TimelineSim cost model):
  * One tiny matmul over a gpsimd-memset scratch tile fires at t~250 to
    anchor the PE p-state ramp (the ramp window survives PE idle, so the
    old 8-matmul warm-up burn is replaced by real work at full clock
    from ~2.4us).
  * All input tensors are staged p-major in DRAM (one descriptor per
    partition), making the gpsimd SWDGE lane ~1us per tensor; the first
    sample's hn rides the HWDGE lane in arrival-ordered chunks and the
    front fill consumes them wave-by-wave (term-major across a quad of
    PSUM tiles).
  * Tail: the last sample's evacuations alternate ACT/DVE, output DMAs
    spread across the sync/scalar/gpsimd queues, and the final tile is
    split in half so the end-of-kernel evac+DMA+semaphore drain chain is
    short.
"""

import numpy as np
import ml_dtypes

import concourse.bass as bass
import concourse.mybir as mybir
import concourse.tile as tile
from concourse import bacc
from concourse.bass_utils import run_bass_kernel_spmd

NCORES = 8
B_FULL, C, H, W = 16, 512, 32, 32
B_LOC = B_FULL // NCORES          # samples per core
N = H * W                         # spatial tokens
G = 32                            # groupnorm groups
EPS = 1e-6
P = 128
NKO = C // P                      # channel chunks (4)
NMM = N // P                      # spatial chunks (8)
NH = 512                          # n-half size
WS = 16.0                         # fp8 pre-scale on M01/W23
C0 = 6.0                          # softmax-invariant exp shift
SEXP = float(C) ** -0.5 / WS      # St psum carries one factor of WS (M01)

F32 = mybir.dt.float32
F8 = mybir.dt.float8e4
F8E5 = mybir.dt.float8e5
E4M3 = ml_dtypes.float8_e4m3
Act = mybir.ActivationFunctionType
DR = mybir.MatmulPerfMode.DoubleRow


def _build(fast_bias):
    nc = bacc.Bacc("TRN2", target_bir_lowering=False, debug=False)

    # p-major DRAM layouts: one contiguous run per partition per chunk.
    hn_d = nc.dram_tensor("hn8", [B_LOC, 2, P, NKO, N], F8,
                          kind="ExternalInput").ap()
    m01_d = nc.dram_tensor("M01", [2, P, NKO, C], F8,
                           kind="ExternalInput").ap()
    w23_d = nc.dram_tensor("W23", [2, P, NKO, C], F8,
                           kind="ExternalInput").ap()
    rho_d = nc.dram_tensor("rho", [B_LOC, P, NMM], F32,
                           kind="ExternalInput").ap()
    num_d = nc.dram_tensor("num", [B_LOC, NMM, P, C], F32,
                           kind="ExternalOutput").ap()
    den_d = nc.dram_tensor("den", [B_LOC, P, NMM], F32,
                           kind="ExternalOutput").ap()

    ones_np = np.full((P, 2, 1), WS, dtype=E4M3)
    ones_d = nc.inline_tensor(ones_np, name="ones16").ap()

    with tile.TileContext(nc) as tc:
        _body(tc, hn_d, m01_d, w23_d, rho_d, ones_d, num_d, den_d, fast_bias)
    nc.compile()
    return nc


def _body(tc, hn_d, m01_d, w23_d, rho_d, ones_d, num_d, den_d, fast_bias):
    nc = tc.nc
    import contextlib

    with contextlib.ExitStack() as ctx:
        singles = ctx.enter_context(tc.tile_pool(name="singles", bufs=1))
        hnpool = ctx.enter_context(tc.tile_pool(name="hnpool", bufs=2))
        gpool = ctx.enter_context(tc.tile_pool(name="gpool", bufs=2))
        vpool = ctx.enter_context(tc.tile_pool(name="vpool", bufs=2))
        espool = ctx.enter_context(tc.tile_pool(name="espool", bufs=2))
        ypool = ctx.enter_context(tc.tile_pool(name="ypool", bufs=4))
        ps = ctx.enter_context(tc.tile_pool(name="ps", bufs=7, space="PSUM"))
        psd = ctx.enter_context(tc.tile_pool(name="psd", bufs=1, space="PSUM"))

        hn_sb = []
        rho_sb = []

        g8 = [None] * B_LOC
        vW8 = [None] * B_LOC
        eS8 = [None] * B_LOC
        den_t = psd.tile([P, B_LOC, NMM], F32, name="den_t", tag="den",
                         space="PSUM")

        # p-state ramp anchor: one tiny DR matmul over a gpsimd-memset
        # scratch tile.  The ramp window is keyed to the FIRST PE matmul
        # and survives idle, so by the time the input DMAs land (~3.2us)
        # the clock is most of the way to full speed.  The garbage result
        # lands in den_t partitions/columns that every den matmul later
        # resets with start=True.
        scr = singles.tile([P, 2, 32], F8, name="scr", tag="scr")
        nc.gpsimd.memset(scr, 0)
        nc.tensor.matmul(den_t[0:32, :, :], scr[:, :, 0:32], scr[:, :, 0:16],
                         start=True, stop=True, perf_mode=DR)

        # ---- input DMAs, priority-ordered ----
        # sync/SP HWDGE lane (byte-rate bound): sample-0 hn in arrival-
        # ordered 128K chunks, then the w23 pair.  Adjacent issues are
        # deliberately non-contiguous so the framework cannot coalesce
        # them (each DMA completion carries a +900ns semaphore delay, so
        # fine granularity is what makes the early chunks usable early).
        t0 = hnpool.tile([P, 2, NKO, N], F8, name="hn_s0", tag="hn")
        hn_sb.append(t0)
        nc.sync.dma_start(t0[:, 0, 0:2, 0:NH], hn_d[0, 0, :, 0:2, 0:NH])
        nc.sync.dma_start(t0[:, 0, 2:4, 0:NH], hn_d[0, 0, :, 2:4, 0:NH])
        nc.sync.dma_start(t0[:, 0, 0:2, NH:N], hn_d[0, 0, :, 0:2, NH:N])
        nc.sync.dma_start(t0[:, 0, 2:4, NH:N], hn_d[0, 0, :, 2:4, NH:N])
        nc.sync.dma_start(t0[:, 1, 0:2, 0:NH], hn_d[0, 1, :, 0:2, 0:NH])
        nc.sync.dma_start(t0[:, 1, 2:4, 0:NH], hn_d[0, 1, :, 2:4, 0:NH])
        nc.sync.dma_start(t0[:, 1, 0:2, NH:N], hn_d[0, 1, :, 0:2, NH:N])
        nc.sync.dma_start(t0[:, 1, 2:4, NH:N], hn_d[0, 1, :, 2:4, NH:N])
        w23_sb = singles.tile([P, 2, NKO, C], F8, name="w23_sb", tag="w23")
        nc.sync.dma_start(w23_sb[:, 0], w23_d[0])
        nc.sync.dma_start(w23_sb[:, 1], w23_d[1])

        # gpsimd/Pool SWDGE lane (descriptor bound; p-major = 1 desc per
        # partition): the m01 pair first (gates the very first matmul
        # together with the first hn chunk), then sample-1 hn, then the
        # small late-needed tensors (keeping them off the shared HWDGE
        # issue slots that the hn chunks need).
        m01_sb = singles.tile([P, 2, NKO, C], F8, name="m01_sb", tag="m01")
        nc.gpsimd.dma_start(m01_sb[:, 0], m01_d[0])
        nc.gpsimd.dma_start(m01_sb[:, 1], m01_d[1])
        t1 = hnpool.tile([P, 2, NKO, N], F8, name="hn_s1", tag="hn")
        hn_sb.append(t1)
        nc.gpsimd.dma_start(t1[:, 0], hn_d[1, 0])
        nc.gpsimd.dma_start(t1[:, 1], hn_d[1, 1])
        ones_sb = singles.tile([P, 2, 1], F8, name="ones_sb", tag="ones")
        nc.gpsimd.dma_start(ones_sb, ones_d)
        r0 = singles.tile([P, NMM], F32, name="rho_s0", tag="rho0")
        nc.gpsimd.dma_start(r0, rho_d[0])
        rho_sb.append(r0)
        r1 = singles.tile([P, NMM], F32, name="rho_s1", tag="rho1")
        nc.gpsimd.dma_start(r1, rho_d[1])
        rho_sb.append(r1)

        # two-term operand pairs (hi*hi, lo*hi, hi*lo; lo*lo dropped at
        # ~0.13% magnitude).
        TERMS = ((0, 0), (1, 0), (0, 1))

        def evac2(dst_hi, dst_lo, pt):
            """PSUM -> two-term fp8: hi on ACT, lo (residual) on DVE."""
            nc.scalar.activation(dst_hi, pt, Act.Identity)
            nc.vector.tensor_tensor(dst_lo, pt, dst_hi,
                                    mybir.AluOpType.subtract)

        def front(s):
            """g = (16 M01)^T hn and vW = hn^T (16 W23): three two-term
            cross products accumulated in PSUM, evacuated to fp8 pairs."""
            hn = hn_sb[s]
            g8[s] = gpool.tile([P, 2, NKO, N], F8, name=f"g8_s{s}", tag="g8")
            vW8[s] = vpool.tile([P, 2, NMM, C], F8, name=f"vW8_s{s}",
                                tag="vW8")

            def g_tile(dc, nh):
                gt = ps.tile([P, NH], F32, name=f"g_{dc}_{nh}_s{s}",
                             tag="big", space="PSUM")
                k = 0
                for tm, th in TERMS:
                    for j in range(2):
                        nc.tensor.matmul(
                            gt, m01_sb[:, tm, 2 * j:2 * j + 2,
                                       dc * P:(dc + 1) * P],
                            hn[:, th, 2 * j:2 * j + 2, nh * NH:(nh + 1) * NH],
                            start=(k == 0), stop=(k == 5), perf_mode=DR)
                        k += 1
                evac2(g8[s][:, 0, dc, nh * NH:(nh + 1) * NH],
                      g8[s][:, 1, dc, nh * NH:(nh + 1) * NH], gt)

            def v_tile(mm):
                vt = ps.tile([P, NH], F32, name=f"v_{mm}_s{s}", tag="big",
                             space="PSUM")
                k = 0
                # hi*Whi, lo*Whi, hi*Wlo: w23-lo is the latest HWDGE
                # arrival, so it comes last.
                for th, tw in ((0, 0), (1, 0), (0, 1)):
                    for j in range(2):
                        nc.tensor.matmul(
                            vt, hn[:, th, 2 * j:2 * j + 2,
                                   mm * P:(mm + 1) * P],
                            w23_sb[:, tw, 2 * j:2 * j + 2, :],
                            start=(k == 0), stop=(k == 5), perf_mode=DR)
                        k += 1
                evac2(vW8[s][:, 0, mm, :], vW8[s][:, 1, mm, :], vt)

            def g_quad(quad, waves):
                """Sweep term-waves across a quad of g tiles in DMA
                arrival order: (tm, th, j) per wave, nh-minor so the
                first half of each hn chunk unblocks the wave start."""
                gts = {}
                for dc, nh in quad:
                    gts[(dc, nh)] = ps.tile([P, NH], F32,
                                            name=f"g_{dc}_{nh}_s{s}",
                                            tag="big", space="PSUM")
                nwave = len(waves)
                for wi, (tm, th, j) in enumerate(waves):
                    for dc, nh in quad:
                        nc.tensor.matmul(
                            gts[(dc, nh)],
                            m01_sb[:, tm, 2 * j:2 * j + 2,
                                   dc * P:(dc + 1) * P],
                            hn[:, th, 2 * j:2 * j + 2,
                               nh * NH:(nh + 1) * NH],
                            start=(wi == 0), stop=(wi == nwave - 1),
                            perf_mode=DR)
                for dc, nh in quad:
                    evac2(g8[s][:, 0, dc, nh * NH:(nh + 1) * NH],
                          g8[s][:, 1, dc, nh * NH:(nh + 1) * NH],
                          gts[(dc, nh)])

            if s == 0:
                # fill window: waves ordered by DMA arrival --
                # hi*hi j0 (hn-hi ko01-nh), hi*hi j1 (hn-hi ko23-nh),
                # lo*hi j0/j1 (m01-lo via SWDGE), hi*lo j0 (hn-lo ko01),
                # hi*lo j1 (hn-lo ko23).  Each quad holds nh fixed so it
                # only needs that half's chunks.
                waves = ((0, 0, 0), (0, 0, 1), (1, 0, 0),
                         (1, 0, 1), (0, 1, 0), (0, 1, 1))
                g_quad([(dc, 0) for dc in range(4)], waves)
                g_quad([(dc, 1) for dc in range(4)], waves)
                for u in range(8):
                    v_tile(u)
            else:
                # interleave g and vW tiles so the evacuation engines see
                # a steady stream instead of end-of-phase bursts
                for u in range(8):
                    g_tile(u // 2, u % 2)
                    v_tile(u)

        def scores(s, nh):
            """St = hn^T g8 (two-term both sides) for one n-half; exp ->
            eS8 (fp8 e5m2), key-side bias + overflow shift via rho."""
            hn = hn_sb[s]
            if eS8[s] is None:
                eS8[s] = espool.tile([P, NMM, N], F8E5, name=f"eS_s{s}",
                                     tag="eS")
            sl = slice(nh * NH, (nh + 1) * NH)
            for mm in range(NMM):
                st = ps.tile([P, NH], F32, name=f"st_{mm}_{nh}_s{s}",
                             tag="big", space="PSUM")
                k = 0
                for th, tg in TERMS:
                    for j in range(2):
                        nc.tensor.matmul(
                            st, hn[:, th, 2 * j:2 * j + 2,
                                   mm * P:(mm + 1) * P],
                            g8[s][:, tg, 2 * j:2 * j + 2, sl],
                            start=(k == 0), stop=(k == 5), perf_mode=DR)
                        k += 1
                nc.scalar.activation(eS8[s][:, mm, sl], st, Act.Exp,
                                     scale=SEXP, bias=rho_sb[s][:, mm:mm + 1])

        def tail(s, nh):
            """den columns (first, so den leaves early) + numerator
            matmuls (two-term vW) for one n-half; PSUM -> SBUF -> DMA."""
            eS = eS8[s]
            for nck in range(nh * 4, nh * 4 + 4):
                csl = slice(nck * P, (nck + 1) * P)
                for j in range(4):
                    nc.tensor.matmul(
                        den_t[:, s, nck:nck + 1],
                        eS[:, 2 * j:2 * j + 2, csl], ones_sb,
                        start=(j == 0), stop=(j == 3), perf_mode=DR)
            if nh == 1:
                dsb = singles.tile([P, NMM], F32, name=f"den_sb_s{s}",
                                   tag=f"densb{s}")
                nc.vector.tensor_copy(dsb, den_t[:, s, :])
                if s == 0:
                    nc.gpsimd.dma_start(den_d[s], dsb)
                else:
                    nc.scalar.dma_start(den_d[s], dsb)
            for nck in range(nh * 4, nh * 4 + 4):
                csl = slice(nck * P, (nck + 1) * P)
                nt = ps.tile([P, C], F32, name=f"n_{nck}_s{s}", tag="big",
                             space="PSUM")
                k = 0
                for tw in range(2):
                    for j in range(4):
                        nc.tensor.matmul(
                            nt, eS[:, 2 * j:2 * j + 2, csl],
                            vW8[s][:, tw, 2 * j:2 * j + 2, :],
                            start=(k == 0), stop=(k == 7), perf_mode=DR)
                        k += 1
                if s == 0 or nck < 4:
                    # early tiles ride the otherwise-idle SWDGE lane
                    y = ypool.tile([P, C], F32, name=f"y_{nck}_s{s}",
                                   tag="y")
                    nc.vector.tensor_copy(y, nt)
                    nc.gpsimd.dma_start(num_d[s, nck], y)
                elif nck == 7:
                    # final tile: independent halves evac'd on both
                    # engines and DMA'd on both HWDGE queues so the
                    # end-of-kernel evac+DMA+semaphore chain is short.
                    ya = singles.tile([P, 256], F32, name="y7a", tag="y7a")
                    nc.scalar.activation(ya, nt[:, 0:256], Act.Identity)
                    nc.scalar.dma_start(num_d[s, nck, :, 0:256], ya)
                    yb = singles.tile([P, 256], F32, name="y7b", tag="y7b")
                    nc.vector.tensor_copy(yb, nt[:, 256:])
                    nc.sync.dma_start(num_d[s, nck, :, 256:], yb)
                else:
                    # drain phase: ACT is done with exp here, so
                    # alternate the evac engine and the HWDGE queue
                    y = ypool.tile([P, C], F32, name=f"y_{nck}_s{s}",
                                   tag="y")
                    if nck % 2 == 0:
                        nc.scalar.activation(y, nt, Act.Identity)
                        nc.sync.dma_start(num_d[s, nck], y)
                    else:
                        nc.vector.tensor_copy(y, nt)
                        nc.scalar.dma_start(num_d[s, nck], y)

        # software pipeline: sample-1 front/scores fill PE slack while
        # sample-0's exp (ACT) and evacuations (DVE) drain, and vice versa.
        front(0)
        scores(0, 0)
        scores(0, 1)
        front(1)
        tail(0, 0)
        scores(1, 0)
        tail(0, 1)
        scores(1, 1)
        tail(1, 0)
        tail(1, 1)


_NC_CACHE = {}


def _get_nc(fast_bias=True):
    key = bool(fast_bias)
    if key not in _NC_CACHE:
        _NC_CACHE[key] = _build(key)
    return _NC_CACHE[key]


def _groupnorm_host(x, gamma, beta):
    b, c, h, w = x.shape
    xg = x.reshape(b, G, c // G, h * w)
    mu = xg.mean(axis=(2, 3), keepdims=True)
    var = xg.var(axis=(2, 3), keepdims=True)
    xn = ((xg - mu) / np.sqrt(var + EPS)).reshape(b, c, h * w)
    return xn * gamma[None, :, None] + beta[None, :, None]


def run(inputs, trace=False):
    f64 = np.float64
    W0 = np.asarray(inputs["W0"], f64)
    W1 = np.asarray(inputs["W1"], f64)
    W2 = np.asarray(inputs["W2"], f64)
    W3 = np.asarray(inputs["W3"], f64)
    b0 = np.asarray(inputs["b0"], f64)
    b2 = np.asarray(inputs["b2"], f64)
    b3 = np.asarray(inputs["b3"], f64)

    x = np.asarray(inputs["x"], np.float32)
    gamma = np.asarray(inputs["gn_gamma"], np.float32)
    beta = np.asarray(inputs["gn_beta"], np.float32)

    hn = _groupnorm_host(x, gamma, beta)              # [B, C, N] f32
    hn_hi = hn.astype(E4M3)
    hn_lo = (hn - hn_hi.astype(np.float32)).astype(E4M3)
    hn8 = np.stack([hn_hi, hn_lo], axis=1)            # [B, 2, C, N]
    # p-major: [B, 2, P, NKO, N]
    hn8 = np.ascontiguousarray(
        hn8.reshape(B_FULL, 2, NKO, P, N).transpose(0, 1, 3, 2, 4))

    M01 = (W0 @ W1.T) * WS
    W23 = (W2 @ W3) * WS
    b23 = (W3.T @ b2 + b3).astype(np.float32)
    r1 = W1 @ b0

    fast_bias = not np.any(r1)
    s = float(C) ** -0.5
    if fast_bias:
        rho = np.full((B_FULL, N), -C0, np.float32)
    else:
        # key-side bias of q.k, shifted per sample so exp() stays in the
        # fp8 range; the shift is softmax-invariant.
        rho = s * np.einsum("c,bcn->bn", r1, hn.astype(f64))
        rho = (rho - np.maximum(rho.max(axis=1, keepdims=True), 0.0)
               - C0).astype(np.float32)
    # p-major: [B, P, NMM]
    rho_pm = np.ascontiguousarray(
        rho.reshape(B_FULL, NMM, P).transpose(0, 2, 1))

    nc = _get_nc(fast_bias)

    def two_term(a):
        a = a.astype(np.float32)
        hi = a.astype(E4M3)
        lo = (a - hi.astype(np.float32)).astype(E4M3)
        pair = np.stack([hi, lo], axis=0)             # [2, C, C]
        # p-major: [2, P, NKO, C]
        return np.ascontiguousarray(
            pair.reshape(2, NKO, P, C).transpose(0, 2, 1, 3))

    base = {
        "M01": two_term(M01),
        "W23": two_term(W23),
    }
    in_maps = []
    for cid in range(NCORES):
        sl = slice(cid * B_LOC, (cid + 1) * B_LOC)
        in_maps.append(dict(base,
                            hn8=np.ascontiguousarray(hn8[sl]),
                            rho=np.ascontiguousarray(rho_pm[sl])))
    res = run_bass_kernel_spmd(nc, in_maps, list(range(NCORES)), trace=trace)

    num = np.concatenate([r["num"] for r in res.results], axis=0)
    den = np.concatenate([r["den"] for r in res.results], axis=0)
    # num[b, nck, p, d]: n = nck*128 + p ; den[b, p, nc]: n = nc*128 + p
    num = num.reshape(B_FULL, N, C)
    den = den.transpose(0, 2, 1).reshape(B_FULL, N)
    o = num / den[:, :, None]                          # [B, N, C]
    out = x + b23[None, :, None, None] \
        + o.transpose(0, 2, 1).reshape(B_FULL, C, H, W).astype(np.float32)
    return out, res


def kernel(**inputs) -> np.ndarray:
    out, _ = run(inputs)
    return out


# revision 11
# speedup vs baseline: 1.0294x; 1.0149x over previous
"""AttnBlockpp (GroupNorm -> q/k/v NIN -> full spatial attention -> NIN ->
residual) for Trainium2, data-parallel over batch across 8 NeuronCores.
Per-core shard: 2 samples of [512, 32, 32] (N = 1024 spatial tokens).

Host-side pre/post-processing (extends the weight folding the original
baseline shipped with):

    M01 = W0 @ W1^T             scores[m,n] = hn_m^T M01^T hn_n + r1.hn_m
    W23 = W2 @ W3               o[n,:] = attn-avg over keys m of (hn^T W23)[m,:]
    b23 = W3^T b2 + b3          r1  = W1 @ b0
    hn  = groupnorm(x)          (exact f32 stats, as the reference)
    epilogue: out = x + b23 + num/den   (softmax normalizer + residual)

The query-side b1 term and the b0.b1 constant cancel inside the softmax
over keys m; the key-side term r1.hn_m rides the exp() bias together with
a softmax-invariant shift C0 that keeps exp() inside the fp8 range.

All four large matmuls run as fp8 DoubleRow (two 128-deep k-tiles per
instruction at 0.5 PE cycles/row = 4x the fp32r/bf16 rate).  Plain e4m3
operands are too noisy for the 2e-2 gate, so every operand is carried as
a TWO-TERM e4m3 pair T = hi + lo (lo = fp8(T - hi), ~0.13% effective
error) and each product keeps three cross terms (hi*hi, lo*hi, hi*lo).
eS = exp(scores - C0) is stored once in e5m2 - its 22-nat range covers
the heavy-tailed scores where e4m3's 11.7-nat window cannot, and its 7%
weight noise is self-cancelling for peaked softmax rows because num and
den use the same quantized eS.

Per sample on the device (PSUM f32 accumulation; WS=16 pre-scale on
M01/W23 puts their entries in the e4m3 normal range and cancels between
the exp scale, the 16.0-valued ones vector of den, and num/den):

    g   [d,n] = (16 M01)^T hn    48 DR matmuls -> ACT hi / DVE lo -> g8 pair
    vW  [m,d] = hn^T (16 W23)    48 DR matmuls -> ACT hi / DVE lo -> vW8 pair
    St  [m,n] = hn^T g8          96 DR matmuls
    eS  [m,n] = exp(St*s + rho[m])   ACT Exp -> e5m2
    den [n]   = 16 sum_m eS      32 free DR matmuls vs ones16
    num [n,d] = eS^T (vW8 hi+lo) 64 DR matmuls -> evac -> DMA (f32)

Schedule notes (v2, tuned against the TimelineSim cost model):
  * One tiny matmul over a gpsimd-memset scratch tile fires at t~250 to
    anchor the PE p-state ramp (the ramp window survives PE idle, so the
    old 8-matmul warm-up burn is replaced by real work at full clock
    from ~2.4us).
  * All input tensors are staged p-major in DRAM (one descriptor per
    partition), making the gpsimd SWDGE lane ~1us per tensor; the first
    sample's hn rides the HWDGE lane in arrival-ordered chunks and the
    front fill consumes them wave-by-wave (term-major across a quad of
    PSUM tiles).
  * Tail: the last sample's evacuations alternate ACT/DVE, output DMAs
    spread across the sync/scalar/gpsimd queues, and the final tile is
    split in half so the end-of-kernel evac+DMA+semaphore drain chain is
    short.
"""

import numpy as np
import ml_dtypes

import concourse.bass as bass
import concourse.mybir as mybir
import concourse.tile as tile
from concourse import bacc
from concourse.bass_utils import run_bass_kernel_spmd

NCORES = 8
B_FULL, C, H, W = 16, 512, 32, 32
B_LOC = B_FULL // NCORES          # samples per core
N = H * W                         # spatial tokens
G = 32                            # groupnorm groups
EPS = 1e-6
P = 128
NKO = C // P                      # channel chunks (4)
NMM = N // P                      # spatial chunks (8)
NH = 512                          # n-half size
WS = 16.0                         # fp8 pre-scale on M01/W23
C0 = 6.0                          # softmax-invariant exp shift
SEXP = float(C) ** -0.5 / WS      # St psum carries one factor of WS (M01)

F32 = mybir.dt.float32
F8 = mybir.dt.float8e4
F8E5 = mybir.dt.float8e5
E4M3 = ml_dtypes.float8_e4m3
Act = mybir.ActivationFunctionType
DR = mybir.MatmulPerfMode.DoubleRow


def _build(fast_bias):
    nc = bacc.Bacc("TRN2", target_bir_lowering=False, debug=False)

    # p-major DRAM layouts: one contiguous run per partition per chunk.
    hn_d = nc.dram_tensor("hn8", [B_LOC, 2, P, NKO, N], F8,
                          kind="ExternalInput").ap()
    m01_d = nc.dram_tensor("M01", [2, P, NKO, C], F8,
                           kind="ExternalInput").ap()
    w23_d = nc.dram_tensor("W23", [2, P, NKO, C], F8,
                           kind="ExternalInput").ap()
    rho_d = nc.dram_tensor("rho", [B_LOC, P, NMM], F32,
                           kind="ExternalInput").ap()
    num_d = nc.dram_tensor("num", [B_LOC, NMM, P, C], F32,
                           kind="ExternalOutput").ap()
    den_d = nc.dram_tensor("den", [B_LOC, P, NMM], F32,
                           kind="ExternalOutput").ap()

    ones_np = np.full((P, 2, 1), WS, dtype=E4M3)
    ones_d = nc.inline_tensor(ones_np, name="ones16").ap()

    with tile.TileContext(nc) as tc:
        _body(tc, hn_d, m01_d, w23_d, rho_d, ones_d, num_d, den_d, fast_bias)
    nc.compile()
    return nc


def _body(tc, hn_d, m01_d, w23_d, rho_d, ones_d, num_d, den_d, fast_bias):
    nc = tc.nc
    import contextlib

    with contextlib.ExitStack() as ctx:
        singles = ctx.enter_context(tc.tile_pool(name="singles", bufs=1))
        hnpool = ctx.enter_context(tc.tile_pool(name="hnpool", bufs=2))
        gpool = ctx.enter_context(tc.tile_pool(name="gpool", bufs=2))
        vpool = ctx.enter_context(tc.tile_pool(name="vpool", bufs=2))
        espool = ctx.enter_context(tc.tile_pool(name="espool", bufs=2))
        ypool = ctx.enter_context(tc.tile_pool(name="ypool", bufs=4))
        ps = ctx.enter_context(tc.tile_pool(name="ps", bufs=7, space="PSUM"))
        psd = ctx.enter_context(tc.tile_pool(name="psd", bufs=1, space="PSUM"))

        hn_sb = []
        rho_sb = []

        g8 = [None] * B_LOC
        vW8 = [None] * B_LOC
        eS8 = [None] * B_LOC
        den_t = psd.tile([P, B_LOC, NMM], F32, name="den_t", tag="den",
                         space="PSUM")

        # p-state ramp anchor: one tiny DR matmul over a gpsimd-memset
        # scratch tile.  The ramp window is keyed to the FIRST PE matmul
        # and survives idle, so by the time the input DMAs land (~3.2us)
        # the clock is most of the way to full speed.  The garbage result
        # lands in den_t partitions/columns that every den matmul later
        # resets with start=True.
        scr = singles.tile([P, 2, 32], F8, name="scr", tag="scr")
        nc.gpsimd.memset(scr, 0)
        nc.tensor.matmul(den_t[0:32, :, :], scr[:, :, 0:32], scr[:, :, 0:16],
                         start=True, stop=True, perf_mode=DR)

        # ---- input DMAs ----
        # Every DMA's bytes flow through one shared 360B/ns engine FIFO
        # in readiness order, HWDGE issues serialize at ~650ns apiece,
        # and each completion semaphore costs +900ns.  So: the sync lane
        # carries sample-0's tensors as pieces ordered EXACTLY by first
        # consumption, and the Pool/SWDGE lane (whose descriptor
        # generation is a parallel issue path) is held back by a delay
        # memset so sample-1's bytes do not jump the FIFO ahead of the
        # critical sample-0 pieces.
        t0 = hnpool.tile([P, 2, NKO, N], F8, name="hn_s0", tag="hn")
        hn_sb.append(t0)
        m01_sb = singles.tile([P, 2, NKO, C], F8, name="m01_sb", tag="m01")
        w23_sb = singles.tile([P, 2, NKO, C], F8, name="w23_sb", tag="w23")
        nc.sync.dma_start(m01_sb[:, 0, 0:2, :], m01_d[0, :, 0:2, :])
        nc.sync.dma_start(t0[:, 0, :, 0:NH], hn_d[0, 0, :, :, 0:NH])
        nc.sync.dma_start(m01_sb[:, 0, 2:4, :], m01_d[0, :, 2:4, :])
        nc.sync.dma_start(t0[:, 0, :, NH:N], hn_d[0, 0, :, :, NH:N])
        nc.sync.dma_start(t0[:, 1, :, 0:NH], hn_d[0, 1, :, :, 0:NH])
        nc.sync.dma_start(t0[:, 1, :, NH:N], hn_d[0, 1, :, :, NH:N])
        nc.sync.dma_start(m01_sb[:, 1], m01_d[1])
        nc.sync.dma_start(w23_sb[:, 0], w23_d[0])
        nc.sync.dma_start(w23_sb[:, 1], w23_d[1])

        # Pool/SWDGE lane: a ~5.5us delay memset keeps its bytes out of
        # the early FIFO, then sample-1 hn and the small late tensors.
        delay = singles.tile([P, 6600], F8, name="delay", tag="delay")
        nc.gpsimd.memset(delay, 0)
        t1 = hnpool.tile([P, 2, NKO, N], F8, name="hn_s1", tag="hn")
        hn_sb.append(t1)
        nc.gpsimd.dma_start(t1[:, 0], hn_d[1, 0])
        nc.gpsimd.dma_start(t1[:, 1], hn_d[1, 1])
        ones_sb = singles.tile([P, 2, 1], F8, name="ones_sb", tag="ones")
        nc.gpsimd.dma_start(ones_sb, ones_d)
        r0 = singles.tile([P, NMM], F32, name="rho_s0", tag="rho0")
        nc.gpsimd.dma_start(r0, rho_d[0])
        rho_sb.append(r0)
        r1 = singles.tile([P, NMM], F32, name="rho_s1", tag="rho1")
        nc.gpsimd.dma_start(r1, rho_d[1])
        rho_sb.append(r1)

        # two-term operand pairs (hi*hi, lo*hi, hi*lo; lo*lo dropped at
        # ~0.13% magnitude).
        TERMS = ((0, 0), (1, 0), (0, 1))

        def evac2(dst_hi, dst_lo, pt):
            """PSUM -> two-term fp8: hi on ACT, lo (residual) on DVE."""
            nc.scalar.activation(dst_hi, pt, Act.Identity)
            nc.vector.tensor_tensor(dst_lo, pt, dst_hi,
                                    mybir.AluOpType.subtract)

        def front(s):
            """g = (16 M01)^T hn and vW = hn^T (16 W23): three two-term
            cross products accumulated in PSUM, evacuated to fp8 pairs."""
            hn = hn_sb[s]
            g8[s] = gpool.tile([P, 2, NKO, N], F8, name=f"g8_s{s}", tag="g8")
            vW8[s] = vpool.tile([P, 2, NMM, C], F8, name=f"vW8_s{s}",
                                tag="vW8")

            def g_tile(dc, nh):
                gt = ps.tile([P, NH], F32, name=f"g_{dc}_{nh}_s{s}",
                             tag="big", space="PSUM")
                k = 0
                for tm, th in TERMS:
                    for j in range(2):
                        nc.tensor.matmul(
                            gt, m01_sb[:, tm, 2 * j:2 * j + 2,
                                       dc * P:(dc + 1) * P],
                            hn[:, th, 2 * j:2 * j + 2, nh * NH:(nh + 1) * NH],
                            start=(k == 0), stop=(k == 5), perf_mode=DR)
                        k += 1
                evac2(g8[s][:, 0, dc, nh * NH:(nh + 1) * NH],
                      g8[s][:, 1, dc, nh * NH:(nh + 1) * NH], gt)

            def v_tile(mm):
                vt = ps.tile([P, NH], F32, name=f"v_{mm}_s{s}", tag="big",
                             space="PSUM")
                k = 0
                # hi*Whi, lo*Whi, hi*Wlo: w23-lo is the latest HWDGE
                # arrival, so it comes last.
                for th, tw in ((0, 0), (1, 0), (0, 1)):
                    for j in range(2):
                        nc.tensor.matmul(
                            vt, hn[:, th, 2 * j:2 * j + 2,
                                   mm * P:(mm + 1) * P],
                            w23_sb[:, tw, 2 * j:2 * j + 2, :],
                            start=(k == 0), stop=(k == 5), perf_mode=DR)
                        k += 1
                evac2(vW8[s][:, 0, mm, :], vW8[s][:, 1, mm, :], vt)

            if s == 0:
                # fill window: seven PSUM banks hold 4 nh0-tiles (A) and
                # 3 nh1-tiles (B); term-waves sweep them interleaved in
                # DMA arrival order -- hi*hi (hn-hi nh0 then nh1),
                # hi*lo (hn-lo), lo*hi (m01-lo, the last sync piece
                # before w23).  The leftover (3,1) tile runs solo on a
                # bank freed by the A evacuations.
                waves = ((0, 0, 0), (0, 0, 1), (0, 1, 0),
                         (0, 1, 1), (1, 0, 0), (1, 0, 1))
                tilesA = [(dc, 0) for dc in range(4)]
                tilesB = [(0, 1), (1, 1), (2, 1)]
                gts = {}
                for dc, nh in tilesA + tilesB:
                    gts[(dc, nh)] = ps.tile([P, NH], F32,
                                            name=f"g_{dc}_{nh}_s{s}",
                                            tag="big", space="PSUM")

                def g_wave(tiles, wi):
                    tm, th, j = waves[wi]
                    for dc, nh in tiles:
                        nc.tensor.matmul(
                            gts[(dc, nh)],
                            m01_sb[:, tm, 2 * j:2 * j + 2,
                                   dc * P:(dc + 1) * P],
                            hn[:, th, 2 * j:2 * j + 2,
                               nh * NH:(nh + 1) * NH],
                            start=(wi == 0), stop=(wi == 5),
                            perf_mode=DR)

                def g_evac(tiles):
                    for dc, nh in tiles:
                        evac2(g8[s][:, 0, dc, nh * NH:(nh + 1) * NH],
                              g8[s][:, 1, dc, nh * NH:(nh + 1) * NH],
                              gts[(dc, nh)])

                g_wave(tilesA, 0)
                g_wave(tilesA, 1)
                g_wave(tilesB, 0)
                g_wave(tilesB, 1)
                g_wave(tilesA, 2)
                g_wave(tilesA, 3)
                g_wave(tilesB, 2)
                g_wave(tilesB, 3)
                g_wave(tilesA, 4)
                g_wave(tilesA, 5)
                g_evac(tilesA)
                g_wave(tilesB, 4)
                g_wave(tilesB, 5)
                g_evac(tilesB)
                g_tile(3, 1)
                for u in range(8):
                    v_tile(u)
            else:
                # interleave g and vW tiles so the evacuation engines see
                # a steady stream instead of end-of-phase bursts
                for u in range(8):
                    g_tile(u // 2, u % 2)
                    v_tile(u)

        def scores(s, nh):
            """St = hn^T g8 (two-term both sides) for one n-half; exp ->
            eS8 (fp8 e5m2), key-side bias + overflow shift via rho."""
            hn = hn_sb[s]
            if eS8[s] is None:
                eS8[s] = espool.tile([P, NMM, N], F8E5, name=f"eS_s{s}",
                                     tag="eS")
            sl = slice(nh * NH, (nh + 1) * NH)
            for mm in range(NMM):
                st = ps.tile([P, NH], F32, name=f"st_{mm}_{nh}_s{s}",
                             tag="big", space="PSUM")
                k = 0
                for th, tg in TERMS:
                    for j in range(2):
                        nc.tensor.matmul(
                            st, hn[:, th, 2 * j:2 * j + 2,
                                   mm * P:(mm + 1) * P],
                            g8[s][:, tg, 2 * j:2 * j + 2, sl],
                            start=(k == 0), stop=(k == 5), perf_mode=DR)
                        k += 1
                nc.scalar.activation(eS8[s][:, mm, sl], st, Act.Exp,
                                     scale=SEXP, bias=rho_sb[s][:, mm:mm + 1])

        def tail(s, nh):
            """den columns (first, so den leaves early) + numerator
            matmuls (two-term vW) for one n-half; PSUM -> SBUF -> DMA."""
            eS = eS8[s]
            for nck in range(nh * 4, nh * 4 + 4):
                csl = slice(nck * P, (nck + 1) * P)
                for j in range(4):
                    nc.tensor.matmul(
                        den_t[:, s, nck:nck + 1],
                        eS[:, 2 * j:2 * j + 2, csl], ones_sb,
                        start=(j == 0), stop=(j == 3), perf_mode=DR)
            if nh == 1:
                dsb = singles.tile([P, NMM], F32, name=f"den_sb_s{s}",
                                   tag=f"densb{s}")
                nc.vector.tensor_copy(dsb, den_t[:, s, :])
                nc.gpsimd.dma_start(den_d[s], dsb)
            for nck in range(nh * 4, nh * 4 + 4):
                csl = slice(nck * P, (nck + 1) * P)
                nt = ps.tile([P, C], F32, name=f"n_{nck}_s{s}", tag="big",
                             space="PSUM")
                k = 0
                for tw in range(2):
                    for j in range(4):
                        nc.tensor.matmul(
                            nt, eS[:, 2 * j:2 * j + 2, csl],
                            vW8[s][:, tw, 2 * j:2 * j + 2, :],
                            start=(k == 0), stop=(k == 7), perf_mode=DR)
                        k += 1
                if s == 0 or nck < 4:
                    # early tiles ride the otherwise-idle SWDGE lane
                    y = ypool.tile([P, C], F32, name=f"y_{nck}_s{s}",
                                   tag="y")
                    nc.vector.tensor_copy(y, nt)
                    nc.gpsimd.dma_start(num_d[s, nck], y)
                elif nck == 7:
                    # final tile: independent halves evac'd on both
                    # engines, each DMA'd from that engine's own queue
                    # (it has no later work, so the issue cannot block
                    # anything) -- the end drain chain is short.
                    ya = singles.tile([P, 256], F32, name="y7a", tag="y7a")
                    nc.scalar.activation(ya, nt[:, 0:256], Act.Identity)
                    nc.scalar.dma_start(num_d[s, nck, :, 0:256], ya)
                    yb = singles.tile([P, 256], F32, name="y7b", tag="y7b")
                    nc.vector.tensor_copy(yb, nt[:, 256:])
                    nc.sync.dma_start(num_d[s, nck, :, 256:], yb)
                else:
                    # drain phase: ACT is done with exp here, so
                    # alternate the evac engine; all DMAs go on the
                    # sync queue (SP has no engine work, so its issue
                    # serialization cannot stall an evac engine).
                    y = ypool.tile([P, C], F32, name=f"y_{nck}_s{s}",
                                   tag="y")
                    if nck % 2 == 0:
                        nc.scalar.activation(y, nt, Act.Identity)
                    else:
                        nc.vector.tensor_copy(y, nt)
                    nc.sync.dma_start(num_d[s, nck], y)

        # software pipeline: sample-1 front/scores fill PE slack while
        # sample-0's exp (ACT) and evacuations (DVE) drain, and vice versa.
        front(0)
        scores(0, 0)
        scores(0, 1)
        front(1)
        tail(0, 0)
        scores(1, 0)
        tail(0, 1)
        scores(1, 1)
        tail(1, 0)
        tail(1, 1)


_NC_CACHE = {}


def _get_nc(fast_bias=True):
    key = bool(fast_bias)
    if key not in _NC_CACHE:
        _NC_CACHE[key] = _build(key)
    return _NC_CACHE[key]


def _groupnorm_host(x, gamma, beta):
    b, c, h, w = x.shape
    xg = x.reshape(b, G, c // G, h * w)
    mu = xg.mean(axis=(2, 3), keepdims=True)
    var = xg.var(axis=(2, 3), keepdims=True)
    xn = ((xg - mu) / np.sqrt(var + EPS)).reshape(b, c, h * w)
    return xn * gamma[None, :, None] + beta[None, :, None]


def run(inputs, trace=False):
    f64 = np.float64
    W0 = np.asarray(inputs["W0"], f64)
    W1 = np.asarray(inputs["W1"], f64)
    W2 = np.asarray(inputs["W2"], f64)
    W3 = np.asarray(inputs["W3"], f64)
    b0 = np.asarray(inputs["b0"], f64)
    b2 = np.asarray(inputs["b2"], f64)
    b3 = np.asarray(inputs["b3"], f64)

    x = np.asarray(inputs["x"], np.float32)
    gamma = np.asarray(inputs["gn_gamma"], np.float32)
    beta = np.asarray(inputs["gn_beta"], np.float32)

    hn = _groupnorm_host(x, gamma, beta)              # [B, C, N] f32
    hn_hi = hn.astype(E4M3)
    hn_lo = (hn - hn_hi.astype(np.float32)).astype(E4M3)
    hn8 = np.stack([hn_hi, hn_lo], axis=1)            # [B, 2, C, N]
    # p-major: [B, 2, P, NKO, N]
    hn8 = np.ascontiguousarray(
        hn8.reshape(B_FULL, 2, NKO, P, N).transpose(0, 1, 3, 2, 4))

    M01 = (W0 @ W1.T) * WS
    W23 = (W2 @ W3) * WS
    b23 = (W3.T @ b2 + b3).astype(np.float32)
    r1 = W1 @ b0

    fast_bias = not np.any(r1)
    s = float(C) ** -0.5
    if fast_bias:
        rho = np.full((B_FULL, N), -C0, np.float32)
    else:
        # key-side bias of q.k, shifted per sample so exp() stays in the
        # fp8 range; the shift is softmax-invariant.
        rho = s * np.einsum("c,bcn->bn", r1, hn.astype(f64))
        rho = (rho - np.maximum(rho.max(axis=1, keepdims=True), 0.0)
               - C0).astype(np.float32)
    # p-major: [B, P, NMM]
    rho_pm = np.ascontiguousarray(
        rho.reshape(B_FULL, NMM, P).transpose(0, 2, 1))

    nc = _get_nc(fast_bias)

    def two_term(a):
        a = a.astype(np.float32)
        hi = a.astype(E4M3)
        lo = (a - hi.astype(np.float32)).astype(E4M3)
        pair = np.stack([hi, lo], axis=0)             # [2, C, C]
        # p-major: [2, P, NKO, C]
        return np.ascontiguousarray(
            pair.reshape(2, NKO, P, C).transpose(0, 2, 1, 3))

    base = {
        "M01": two_term(M01),
        "W23": two_term(W23),
    }
    in_maps = []
    for cid in range(NCORES):
        sl = slice(cid * B_LOC, (cid + 1) * B_LOC)
        in_maps.append(dict(base,
                            hn8=np.ascontiguousarray(hn8[sl]),
                            rho=np.ascontiguousarray(rho_pm[sl])))
    res = run_bass_kernel_spmd(nc, in_maps, list(range(NCORES)), trace=trace)

    num = np.concatenate([r["num"] for r in res.results], axis=0)
    den = np.concatenate([r["den"] for r in res.results], axis=0)
    # num[b, nck, p, d]: n = nck*128 + p ; den[b, p, nc]: n = nc*128 + p
    num = num.reshape(B_FULL, N, C)
    den = den.transpose(0, 2, 1).reshape(B_FULL, N)
    o = num / den[:, :, None]                          # [B, N, C]
    out = x + b23[None, :, None, None] \
        + o.transpose(0, 2, 1).reshape(B_FULL, C, H, W).astype(np.float32)
    return out, res


def kernel(**inputs) -> np.ndarray:
    out, _ = run(inputs)
    return out


# revision 13
# speedup vs baseline: 1.0366x; 1.0070x over previous
"""AttnBlockpp (GroupNorm -> q/k/v NIN -> full spatial attention -> NIN ->
residual) for Trainium2, data-parallel over batch across 8 NeuronCores.
Per-core shard: 2 samples of [512, 32, 32] (N = 1024 spatial tokens).

Host-side pre/post-processing (extends the weight folding the original
baseline shipped with):

    M01 = W0 @ W1^T             scores[m,n] = hn_m^T M01^T hn_n + r1.hn_m
    W23 = W2 @ W3               o[n,:] = attn-avg over keys m of (hn^T W23)[m,:]
    b23 = W3^T b2 + b3          r1  = W1 @ b0
    hn  = groupnorm(x)          (exact f32 stats, as the reference)
    epilogue: out = x + b23 + num/den   (softmax normalizer + residual)

The query-side b1 term and the b0.b1 constant cancel inside the softmax
over keys m; the key-side term r1.hn_m rides the exp() bias together with
a softmax-invariant shift C0 that keeps exp() inside the fp8 range.

All four large matmuls run as fp8 DoubleRow (two 128-deep k-tiles per
instruction at 0.5 PE cycles/row = 4x the fp32r/bf16 rate).  Plain e4m3
operands are too noisy for the 2e-2 gate, so every operand is carried as
a TWO-TERM e4m3 pair T = hi + lo (lo = fp8(T - hi), ~0.13% effective
error) and each product keeps three cross terms (hi*hi, lo*hi, hi*lo).
eS = exp(scores - C0) is stored once in e5m2 - its 22-nat range covers
the heavy-tailed scores where e4m3's 11.7-nat window cannot, and its 7%
weight noise is self-cancelling for peaked softmax rows because num and
den use the same quantized eS.

Per sample on the device (PSUM f32 accumulation; WS=16 pre-scale on
M01/W23 puts their entries in the e4m3 normal range and cancels between
the exp scale, the 16.0-valued ones vector of den, and num/den):

    g   [d,n] = (16 M01)^T hn    48 DR matmuls -> ACT hi / DVE lo -> g8 pair
    vW  [m,d] = hn^T (16 W23)    48 DR matmuls -> ACT hi / DVE lo -> vW8 pair
    St  [m,n] = hn^T g8          96 DR matmuls
    eS  [m,n] = exp(St*s + rho[m])   ACT Exp -> e5m2
    den [n]   = 16 sum_m eS      32 free DR matmuls vs ones16
    num [n,d] = eS^T (vW8 hi+lo) 64 DR matmuls -> evac -> DMA (f32)

Schedule notes (v2, tuned against the TimelineSim cost model):
  * One tiny matmul over a gpsimd-memset scratch tile fires at t~250 to
    anchor the PE p-state ramp (the ramp window survives PE idle, so the
    old 8-matmul warm-up burn is replaced by real work at full clock
    from ~2.4us).
  * All input tensors are staged p-major in DRAM (one descriptor per
    partition), making the gpsimd SWDGE lane ~1us per tensor; the first
    sample's hn rides the HWDGE lane in arrival-ordered chunks and the
    front fill consumes them wave-by-wave (term-major across a quad of
    PSUM tiles).
  * Tail: the last sample's evacuations alternate ACT/DVE, output DMAs
    spread across the sync/scalar/gpsimd queues, and the final tile is
    split in half so the end-of-kernel evac+DMA+semaphore drain chain is
    short.
"""

import numpy as np
import ml_dtypes

import concourse.bass as bass
import concourse.mybir as mybir
import concourse.tile as tile
from concourse import bacc
from concourse.bass_utils import run_bass_kernel_spmd

NCORES = 8
B_FULL, C, H, W = 16, 512, 32, 32
B_LOC = B_FULL // NCORES          # samples per core
N = H * W                         # spatial tokens
G = 32                            # groupnorm groups
EPS = 1e-6
P = 128
NKO = C // P                      # channel chunks (4)
NMM = N // P                      # spatial chunks (8)
NH = 512                          # n-half size
WS = 16.0                         # fp8 pre-scale on M01/W23
C0 = 6.0                          # softmax-invariant exp shift
SEXP = float(C) ** -0.5 / WS      # St psum carries one factor of WS (M01)

F32 = mybir.dt.float32
F8 = mybir.dt.float8e4
F8E5 = mybir.dt.float8e5
E4M3 = ml_dtypes.float8_e4m3
Act = mybir.ActivationFunctionType
DR = mybir.MatmulPerfMode.DoubleRow


def _build(fast_bias):
    nc = bacc.Bacc("TRN2", target_bir_lowering=False, debug=False)

    # p-major DRAM layouts: one contiguous run per partition per chunk.
    hn_d = nc.dram_tensor("hn8", [B_LOC, 2, P, NKO, N], F8,
                          kind="ExternalInput").ap()
    m01_d = nc.dram_tensor("M01", [2, P, NKO, C], F8,
                           kind="ExternalInput").ap()
    w23_d = nc.dram_tensor("W23", [2, P, NKO, C], F8,
                           kind="ExternalInput").ap()
    rho_d = nc.dram_tensor("rho", [B_LOC, P, NMM], F32,
                           kind="ExternalInput").ap()
    num_d = nc.dram_tensor("num", [B_LOC, NMM, P, C], F32,
                           kind="ExternalOutput").ap()
    den_d = nc.dram_tensor("den", [B_LOC, P, NMM], F32,
                           kind="ExternalOutput").ap()

    ones_np = np.full((P, 2, 1), WS, dtype=E4M3)
    ones_d = nc.inline_tensor(ones_np, name="ones16").ap()

    with tile.TileContext(nc) as tc:
        _body(tc, hn_d, m01_d, w23_d, rho_d, ones_d, num_d, den_d, fast_bias)
    nc.compile()
    return nc


def _body(tc, hn_d, m01_d, w23_d, rho_d, ones_d, num_d, den_d, fast_bias):
    nc = tc.nc
    import contextlib

    with contextlib.ExitStack() as ctx:
        singles = ctx.enter_context(tc.tile_pool(name="singles", bufs=1))
        hnpool = ctx.enter_context(tc.tile_pool(name="hnpool", bufs=2))
        gpool = ctx.enter_context(tc.tile_pool(name="gpool", bufs=2))
        vpool = ctx.enter_context(tc.tile_pool(name="vpool", bufs=2))
        espool = ctx.enter_context(tc.tile_pool(name="espool", bufs=2))
        ypool = ctx.enter_context(tc.tile_pool(name="ypool", bufs=4))
        ps = ctx.enter_context(tc.tile_pool(name="ps", bufs=7, space="PSUM"))
        psd = ctx.enter_context(tc.tile_pool(name="psd", bufs=1, space="PSUM"))

        hn_sb = []
        rho_sb = []

        g8 = [None] * B_LOC
        vW8 = [None] * B_LOC
        eS8 = [None] * B_LOC
        den_t = psd.tile([P, B_LOC, NMM], F32, name="den_t", tag="den",
                         space="PSUM")

        # p-state ramp anchor: one tiny DR matmul over a gpsimd-memset
        # scratch tile.  The ramp window is keyed to the FIRST PE matmul
        # and survives idle, so by the time the input DMAs land (~3.2us)
        # the clock is most of the way to full speed.  The garbage result
        # lands in den_t partitions/columns that every den matmul later
        # resets with start=True.
        scr = singles.tile([P, 2, 32], F8, name="scr", tag="scr")
        nc.gpsimd.memset(scr, 0)
        nc.tensor.matmul(den_t[0:32, :, :], scr[:, :, 0:32], scr[:, :, 0:16],
                         start=True, stop=True, perf_mode=DR)

        # ---- input DMAs ----
        # Every DMA's bytes flow through one shared 360B/ns engine FIFO
        # in readiness order, HWDGE issues serialize at ~650ns apiece,
        # and each completion semaphore costs +900ns.  So: the sync lane
        # carries sample-0's tensors as pieces ordered EXACTLY by first
        # consumption, and the Pool/SWDGE lane (whose descriptor
        # generation is a parallel issue path) is held back by a delay
        # memset so sample-1's bytes do not jump the FIFO ahead of the
        # critical sample-0 pieces.
        t0 = hnpool.tile([P, 2, NKO, N], F8, name="hn_s0", tag="hn")
        hn_sb.append(t0)
        m01_sb = singles.tile([P, 2, NKO, C], F8, name="m01_sb", tag="m01")
        w23_sb = singles.tile([P, 2, NKO, C], F8, name="w23_sb", tag="w23")
        nc.sync.dma_start(t0[:, 0, :, 0:NH], hn_d[0, 0, :, :, 0:NH])
        nc.sync.dma_start(m01_sb[:, 0, 0:2, :], m01_d[0, :, 0:2, :])
        nc.sync.dma_start(m01_sb[:, 0, 2:4, :], m01_d[0, :, 2:4, :])
        nc.sync.dma_start(t0[:, 0, :, NH:N], hn_d[0, 0, :, :, NH:N])
        nc.sync.dma_start(t0[:, 1, :, 0:NH], hn_d[0, 1, :, :, 0:NH])
        nc.sync.dma_start(t0[:, 1, :, NH:N], hn_d[0, 1, :, :, NH:N])
        nc.sync.dma_start(m01_sb[:, 1], m01_d[1])
        nc.sync.dma_start(w23_sb[:, 0], w23_d[0])
        nc.sync.dma_start(w23_sb[:, 1], w23_d[1])

        # Pool/SWDGE lane: a ~5.5us delay memset keeps its bytes out of
        # the early FIFO, then sample-1 hn and the small late tensors.
        delay = singles.tile([P, 6600], F8, name="delay", tag="delay")
        nc.gpsimd.memset(delay, 0)
        t1 = hnpool.tile([P, 2, NKO, N], F8, name="hn_s1", tag="hn")
        hn_sb.append(t1)
        nc.gpsimd.dma_start(t1[:, 0], hn_d[1, 0])
        nc.gpsimd.dma_start(t1[:, 1], hn_d[1, 1])
        ones_sb = singles.tile([P, 2, 1], F8, name="ones_sb", tag="ones")
        nc.gpsimd.dma_start(ones_sb, ones_d)
        r0 = singles.tile([P, NMM], F32, name="rho_s0", tag="rho0")
        nc.gpsimd.dma_start(r0, rho_d[0])
        rho_sb.append(r0)
        r1 = singles.tile([P, NMM], F32, name="rho_s1", tag="rho1")
        nc.gpsimd.dma_start(r1, rho_d[1])
        rho_sb.append(r1)

        # two-term operand pairs (hi*hi, lo*hi, hi*lo; lo*lo dropped at
        # ~0.13% magnitude).
        TERMS = ((0, 0), (1, 0), (0, 1))

        def evac2(dst_hi, dst_lo, pt):
            """PSUM -> two-term fp8: hi on ACT, lo (residual) on DVE."""
            nc.scalar.activation(dst_hi, pt, Act.Identity)
            nc.vector.tensor_tensor(dst_lo, pt, dst_hi,
                                    mybir.AluOpType.subtract)

        def front(s):
            """g = (16 M01)^T hn and vW = hn^T (16 W23): three two-term
            cross products accumulated in PSUM, evacuated to fp8 pairs."""
            hn = hn_sb[s]
            g8[s] = gpool.tile([P, 2, NKO, N], F8, name=f"g8_s{s}", tag="g8")
            vW8[s] = vpool.tile([P, 2, NMM, C], F8, name=f"vW8_s{s}",
                                tag="vW8")

            def g_tile(dc, nh):
                gt = ps.tile([P, NH], F32, name=f"g_{dc}_{nh}_s{s}",
                             tag="big", space="PSUM")
                k = 0
                for tm, th in TERMS:
                    for j in range(2):
                        nc.tensor.matmul(
                            gt, m01_sb[:, tm, 2 * j:2 * j + 2,
                                       dc * P:(dc + 1) * P],
                            hn[:, th, 2 * j:2 * j + 2, nh * NH:(nh + 1) * NH],
                            start=(k == 0), stop=(k == 5), perf_mode=DR)
                        k += 1
                evac2(g8[s][:, 0, dc, nh * NH:(nh + 1) * NH],
                      g8[s][:, 1, dc, nh * NH:(nh + 1) * NH], gt)

            def v_tile(mm):
                vt = ps.tile([P, NH], F32, name=f"v_{mm}_s{s}", tag="big",
                             space="PSUM")
                k = 0
                # hi*Whi, lo*Whi, hi*Wlo: w23-lo is the latest HWDGE
                # arrival, so it comes last.
                for th, tw in ((0, 0), (1, 0), (0, 1)):
                    for j in range(2):
                        nc.tensor.matmul(
                            vt, hn[:, th, 2 * j:2 * j + 2,
                                   mm * P:(mm + 1) * P],
                            w23_sb[:, tw, 2 * j:2 * j + 2, :],
                            start=(k == 0), stop=(k == 5), perf_mode=DR)
                        k += 1
                evac2(vW8[s][:, 0, mm, :], vW8[s][:, 1, mm, :], vt)

            if s == 0:
                # fill window: seven PSUM banks hold 4 nh0-tiles (A) and
                # 3 nh1-tiles (B); term-waves sweep them interleaved in
                # DMA arrival order -- hi*hi (hn-hi nh0 then nh1),
                # hi*lo (hn-lo), lo*hi (m01-lo, the last sync piece
                # before w23).  The leftover (3,1) tile runs solo on a
                # bank freed by the A evacuations.
                waves = ((0, 0, 0), (0, 0, 1), (0, 1, 0),
                         (0, 1, 1), (1, 0, 0), (1, 0, 1))
                tilesA = [(dc, 0) for dc in range(4)]
                tilesB = [(0, 1), (1, 1), (2, 1)]
                gts = {}
                for dc, nh in tilesA + tilesB:
                    gts[(dc, nh)] = ps.tile([P, NH], F32,
                                            name=f"g_{dc}_{nh}_s{s}",
                                            tag="big", space="PSUM")

                def g_wave(tiles, wi):
                    tm, th, j = waves[wi]
                    for dc, nh in tiles:
                        nc.tensor.matmul(
                            gts[(dc, nh)],
                            m01_sb[:, tm, 2 * j:2 * j + 2,
                                   dc * P:(dc + 1) * P],
                            hn[:, th, 2 * j:2 * j + 2,
                               nh * NH:(nh + 1) * NH],
                            start=(wi == 0), stop=(wi == 5),
                            perf_mode=DR)

                def g_evac(tiles):
                    for dc, nh in tiles:
                        evac2(g8[s][:, 0, dc, nh * NH:(nh + 1) * NH],
                              g8[s][:, 1, dc, nh * NH:(nh + 1) * NH],
                              gts[(dc, nh)])

                g_wave(tilesA, 0)
                g_wave(tilesA, 1)
                g_wave(tilesB, 0)
                g_wave(tilesB, 1)
                g_wave(tilesA, 2)
                g_wave(tilesA, 3)
                g_wave(tilesB, 2)
                g_wave(tilesB, 3)
                g_wave(tilesA, 4)
                g_wave(tilesA, 5)
                g_evac(tilesA)
                g_wave(tilesB, 4)
                g_wave(tilesB, 5)
                g_evac(tilesB)
                g_tile(3, 1)
                for u in range(8):
                    v_tile(u)
            else:
                # interleave g and vW tiles so the evacuation engines see
                # a steady stream instead of end-of-phase bursts
                for u in range(8):
                    g_tile(u // 2, u % 2)
                    v_tile(u)

        def scores(s, nh):
            """St = hn^T g8 (two-term both sides) for one n-half; exp ->
            eS8 (fp8 e5m2), key-side bias + overflow shift via rho."""
            hn = hn_sb[s]
            if eS8[s] is None:
                eS8[s] = espool.tile([P, NMM, N], F8E5, name=f"eS_s{s}",
                                     tag="eS")
            sl = slice(nh * NH, (nh + 1) * NH)
            for mm in range(NMM):
                st = ps.tile([P, NH], F32, name=f"st_{mm}_{nh}_s{s}",
                             tag="big", space="PSUM")
                k = 0
                for th, tg in TERMS:
                    for j in range(2):
                        nc.tensor.matmul(
                            st, hn[:, th, 2 * j:2 * j + 2,
                                   mm * P:(mm + 1) * P],
                            g8[s][:, tg, 2 * j:2 * j + 2, sl],
                            start=(k == 0), stop=(k == 5), perf_mode=DR)
                        k += 1
                nc.scalar.activation(eS8[s][:, mm, sl], st, Act.Exp,
                                     scale=SEXP, bias=rho_sb[s][:, mm:mm + 1])

        def tail(s, nh):
            """den columns (first, so den leaves early) + numerator
            matmuls (two-term vW) for one n-half; PSUM -> SBUF -> DMA."""
            eS = eS8[s]
            for nck in range(nh * 4, nh * 4 + 4):
                csl = slice(nck * P, (nck + 1) * P)
                for j in range(4):
                    nc.tensor.matmul(
                        den_t[:, s, nck:nck + 1],
                        eS[:, 2 * j:2 * j + 2, csl], ones_sb,
                        start=(j == 0), stop=(j == 3), perf_mode=DR)
            if nh == 1:
                dsb = singles.tile([P, NMM], F32, name=f"den_sb_s{s}",
                                   tag=f"densb{s}")
                nc.vector.tensor_copy(dsb, den_t[:, s, :])
                nc.gpsimd.dma_start(den_d[s], dsb)
            for nck in range(nh * 4, nh * 4 + 4):
                csl = slice(nck * P, (nck + 1) * P)
                if s == 1 and nck == 7:
                    # final tile: two independently-accumulated column
                    # halves, so the first half's evac+DMA chain starts
                    # while the second half's matmuls still run, and the
                    # very last chain moves only 128KB.
                    for hf in range(2):
                        nt = ps.tile([P, 256], F32, name=f"n_7{hf}_s{s}",
                                     tag="big", space="PSUM")
                        k = 0
                        for tw in range(2):
                            for j in range(4):
                                nc.tensor.matmul(
                                    nt, eS[:, 2 * j:2 * j + 2, csl],
                                    vW8[s][:, tw, 2 * j:2 * j + 2,
                                           hf * 256:(hf + 1) * 256],
                                    start=(k == 0), stop=(k == 7),
                                    perf_mode=DR)
                                k += 1
                        yh = singles.tile([P, 256], F32, name=f"y7{hf}",
                                          tag=f"y7{hf}")
                        if hf == 0:
                            nc.vector.tensor_copy(yh, nt)
                            nc.sync.dma_start(num_d[s, nck, :, 0:256], yh)
                        else:
                            nc.scalar.activation(yh, nt, Act.Identity)
                            nc.scalar.dma_start(num_d[s, nck, :, 256:], yh)
                    continue
                nt = ps.tile([P, C], F32, name=f"n_{nck}_s{s}", tag="big",
                             space="PSUM")
                k = 0
                for tw in range(2):
                    for j in range(4):
                        nc.tensor.matmul(
                            nt, eS[:, 2 * j:2 * j + 2, csl],
                            vW8[s][:, tw, 2 * j:2 * j + 2, :],
                            start=(k == 0), stop=(k == 7), perf_mode=DR)
                        k += 1
                if s == 0 or nck in (2, 3):
                    # spread over the SWDGE lane (idle in the drain)
                    y = ypool.tile([P, C], F32, name=f"y_{nck}_s{s}",
                                   tag="y")
                    nc.vector.tensor_copy(y, nt)
                    nc.gpsimd.dma_start(num_d[s, nck], y)
                else:
                    # sync-queue HWDGE (SP has no engine work, so its
                    # issue serialization cannot stall an evac engine);
                    # ACT takes over evacs once it is done with exp.
                    y = ypool.tile([P, C], F32, name=f"y_{nck}_s{s}",
                                   tag="y")
                    if nh == 1 and nck % 2 == 0:
                        nc.scalar.activation(y, nt, Act.Identity)
                    else:
                        nc.vector.tensor_copy(y, nt)
                    nc.sync.dma_start(num_d[s, nck], y)

        # software pipeline: sample-1 front/scores fill PE slack while
        # sample-0's exp (ACT) and evacuations (DVE) drain, and vice versa.
        front(0)
        scores(0, 0)
        scores(0, 1)
        front(1)
        tail(0, 0)
        scores(1, 0)
        tail(0, 1)
        scores(1, 1)
        tail(1, 0)
        tail(1, 1)


_NC_CACHE = {}


def _get_nc(fast_bias=True):
    key = bool(fast_bias)
    if key not in _NC_CACHE:
        _NC_CACHE[key] = _build(key)
    return _NC_CACHE[key]


def _groupnorm_host(x, gamma, beta):
    b, c, h, w = x.shape
    xg = x.reshape(b, G, c // G, h * w)
    mu = xg.mean(axis=(2, 3), keepdims=True)
    var = xg.var(axis=(2, 3), keepdims=True)
    xn = ((xg - mu) / np.sqrt(var + EPS)).reshape(b, c, h * w)
    return xn * gamma[None, :, None] + beta[None, :, None]


def run(inputs, trace=False):
    f64 = np.float64
    W0 = np.asarray(inputs["W0"], f64)
    W1 = np.asarray(inputs["W1"], f64)
    W2 = np.asarray(inputs["W2"], f64)
    W3 = np.asarray(inputs["W3"], f64)
    b0 = np.asarray(inputs["b0"], f64)
    b2 = np.asarray(inputs["b2"], f64)
    b3 = np.asarray(inputs["b3"], f64)

    x = np.asarray(inputs["x"], np.float32)
    gamma = np.asarray(inputs["gn_gamma"], np.float32)
    beta = np.asarray(inputs["gn_beta"], np.float32)

    hn = _groupnorm_host(x, gamma, beta)              # [B, C, N] f32
    hn_hi = hn.astype(E4M3)
    hn_lo = (hn - hn_hi.astype(np.float32)).astype(E4M3)
    hn8 = np.stack([hn_hi, hn_lo], axis=1)            # [B, 2, C, N]
    # p-major: [B, 2, P, NKO, N]
    hn8 = np.ascontiguousarray(
        hn8.reshape(B_FULL, 2, NKO, P, N).transpose(0, 1, 3, 2, 4))

    M01 = (W0 @ W1.T) * WS
    W23 = (W2 @ W3) * WS
    b23 = (W3.T @ b2 + b3).astype(np.float32)
    r1 = W1 @ b0

    fast_bias = not np.any(r1)
    s = float(C) ** -0.5
    if fast_bias:
        rho = np.full((B_FULL, N), -C0, np.float32)
    else:
        # key-side bias of q.k, shifted per sample so exp() stays in the
        # fp8 range; the shift is softmax-invariant.
        rho = s * np.einsum("c,bcn->bn", r1, hn.astype(f64))
        rho = (rho - np.maximum(rho.max(axis=1, keepdims=True), 0.0)
               - C0).astype(np.float32)
    # p-major: [B, P, NMM]
    rho_pm = np.ascontiguousarray(
        rho.reshape(B_FULL, NMM, P).transpose(0, 2, 1))

    nc = _get_nc(fast_bias)

    def two_term(a):
        a = a.astype(np.float32)
        hi = a.astype(E4M3)
        lo = (a - hi.astype(np.float32)).astype(E4M3)
        pair = np.stack([hi, lo], axis=0)             # [2, C, C]
        # p-major: [2, P, NKO, C]
        return np.ascontiguousarray(
            pair.reshape(2, NKO, P, C).transpose(0, 2, 1, 3))

    base = {
        "M01": two_term(M01),
        "W23": two_term(W23),
    }
    in_maps = []
    for cid in range(NCORES):
        sl = slice(cid * B_LOC, (cid + 1) * B_LOC)
        in_maps.append(dict(base,
                            hn8=np.ascontiguousarray(hn8[sl]),
                            rho=np.ascontiguousarray(rho_pm[sl])))
    res = run_bass_kernel_spmd(nc, in_maps, list(range(NCORES)), trace=trace)

    num = np.concatenate([r["num"] for r in res.results], axis=0)
    den = np.concatenate([r["den"] for r in res.results], axis=0)
    # num[b, nck, p, d]: n = nck*128 + p ; den[b, p, nc]: n = nc*128 + p
    num = num.reshape(B_FULL, N, C)
    den = den.transpose(0, 2, 1).reshape(B_FULL, N)
    o = num / den[:, :, None]                          # [B, N, C]
    out = x + b23[None, :, None, None] \
        + o.transpose(0, 2, 1).reshape(B_FULL, C, H, W).astype(np.float32)
    return out, res


def kernel(**inputs) -> np.ndarray:
    out, _ = run(inputs)
    return out


# revision 14
# speedup vs baseline: 1.0517x; 1.0145x over previous
"""AttnBlockpp (GroupNorm -> q/k/v NIN -> full spatial attention -> NIN ->
residual) for Trainium2, data-parallel over batch across 8 NeuronCores.
Per-core shard: 2 samples of [512, 32, 32] (N = 1024 spatial tokens).

Host-side pre/post-processing (extends the weight folding the original
baseline shipped with):

    M01 = W0 @ W1^T             scores[m,n] = hn_m^T M01^T hn_n + r1.hn_m
    W23 = W2 @ W3               o[n,:] = attn-avg over keys m of (hn^T W23)[m,:]
    b23 = W3^T b2 + b3          r1  = W1 @ b0
    hn  = groupnorm(x)          (exact f32 stats, as the reference)
    epilogue: out = x + b23 + num/den   (softmax normalizer + residual)

The query-side b1 term and the b0.b1 constant cancel inside the softmax
over keys m; the key-side term r1.hn_m rides the exp() bias together with
a softmax-invariant shift C0 that keeps exp() inside the fp8 range.

All four large matmuls run as fp8 DoubleRow (two 128-deep k-tiles per
instruction at 0.5 PE cycles/row = 4x the fp32r/bf16 rate).  Plain e4m3
operands are too noisy for the 2e-2 gate, so every operand is carried as
a TWO-TERM e4m3 pair T = hi + lo (lo = fp8(T - hi), ~0.13% effective
error) and each product keeps three cross terms (hi*hi, lo*hi, hi*lo).
eS = exp(scores - C0) is stored once in e5m2 - its 22-nat range covers
the heavy-tailed scores where e4m3's 11.7-nat window cannot, and its 7%
weight noise is self-cancelling for peaked softmax rows because num and
den use the same quantized eS.

Per sample on the device (PSUM f32 accumulation; WS=16 pre-scale on
M01/W23 puts their entries in the e4m3 normal range and cancels between
the exp scale, the 16.0-valued ones vector of den, and num/den):

    g   [d,n] = (16 M01)^T hn    48 DR matmuls -> ACT hi / DVE lo -> g8 pair
    vW  [m,d] = hn^T (16 W23)    48 DR matmuls -> ACT hi / DVE lo -> vW8 pair
    St  [m,n] = hn^T g8          96 DR matmuls
    eS  [m,n] = exp(St*s + rho[m])   ACT Exp -> e5m2
    den [n]   = 16 sum_m eS      32 free DR matmuls vs ones16
    num [n,d] = eS^T (vW8 hi+lo) 64 DR matmuls -> evac -> DMA (f32)

Schedule notes (v2, tuned against the TimelineSim cost model):
  * One tiny matmul over a gpsimd-memset scratch tile fires at t~250 to
    anchor the PE p-state ramp (the ramp window survives PE idle, so the
    old 8-matmul warm-up burn is replaced by real work at full clock
    from ~2.4us).
  * All input tensors are staged p-major in DRAM (one descriptor per
    partition), making the gpsimd SWDGE lane ~1us per tensor; the first
    sample's hn rides the HWDGE lane in arrival-ordered chunks and the
    front fill consumes them wave-by-wave (term-major across a quad of
    PSUM tiles).
  * Tail: the last sample's evacuations alternate ACT/DVE, output DMAs
    spread across the sync/scalar/gpsimd queues, and the final tile is
    split in half so the end-of-kernel evac+DMA+semaphore drain chain is
    short.
"""

import numpy as np
import ml_dtypes

import concourse.bass as bass
import concourse.mybir as mybir
import concourse.tile as tile
from concourse import bacc
from concourse.bass_utils import run_bass_kernel_spmd

NCORES = 8
B_FULL, C, H, W = 16, 512, 32, 32
B_LOC = B_FULL // NCORES          # samples per core
N = H * W                         # spatial tokens
G = 32                            # groupnorm groups
EPS = 1e-6
P = 128
NKO = C // P                      # channel chunks (4)
NMM = N // P                      # spatial chunks (8)
NH = 512                          # n-half size
WS = 16.0                         # fp8 pre-scale on M01/W23
C0 = 6.0                          # softmax-invariant exp shift
SEXP = float(C) ** -0.5 / WS      # St psum carries one factor of WS (M01)

F32 = mybir.dt.float32
BF16 = mybir.dt.bfloat16
F8 = mybir.dt.float8e4
F8E5 = mybir.dt.float8e5
E4M3 = ml_dtypes.float8_e4m3
Act = mybir.ActivationFunctionType
DR = mybir.MatmulPerfMode.DoubleRow


def _build(fast_bias):
    nc = bacc.Bacc("TRN2", target_bir_lowering=False, debug=False)

    # p-major DRAM layouts: one contiguous run per partition per chunk.
    hn_d = nc.dram_tensor("hn8", [B_LOC, 2, P, NKO, N], F8,
                          kind="ExternalInput").ap()
    m01_d = nc.dram_tensor("M01", [2, P, NKO, C], F8,
                           kind="ExternalInput").ap()
    w23_d = nc.dram_tensor("W23", [2, P, NKO, C], F8,
                           kind="ExternalInput").ap()
    rho_d = nc.dram_tensor("rho", [B_LOC, P, NMM], F32,
                           kind="ExternalInput").ap()
    num_d = nc.dram_tensor("num", [B_LOC, NMM, P, C], BF16,
                           kind="ExternalOutput").ap()
    den_d = nc.dram_tensor("den", [B_LOC, P, NMM], F32,
                           kind="ExternalOutput").ap()

    ones_np = np.full((P, 2, 1), WS, dtype=E4M3)
    ones_d = nc.inline_tensor(ones_np, name="ones16").ap()

    with tile.TileContext(nc) as tc:
        _body(tc, hn_d, m01_d, w23_d, rho_d, ones_d, num_d, den_d, fast_bias)
    nc.compile()
    return nc


def _body(tc, hn_d, m01_d, w23_d, rho_d, ones_d, num_d, den_d, fast_bias):
    nc = tc.nc
    import contextlib

    with contextlib.ExitStack() as ctx:
        singles = ctx.enter_context(tc.tile_pool(name="singles", bufs=1))
        hnpool = ctx.enter_context(tc.tile_pool(name="hnpool", bufs=2))
        gpool = ctx.enter_context(tc.tile_pool(name="gpool", bufs=2))
        vpool = ctx.enter_context(tc.tile_pool(name="vpool", bufs=2))
        espool = ctx.enter_context(tc.tile_pool(name="espool", bufs=2))
        ypool = ctx.enter_context(tc.tile_pool(name="ypool", bufs=6))
        ps = ctx.enter_context(tc.tile_pool(name="ps", bufs=7, space="PSUM"))
        psd = ctx.enter_context(tc.tile_pool(name="psd", bufs=1, space="PSUM"))

        hn_sb = []
        rho_sb = []

        g8 = [None] * B_LOC
        vW8 = [None] * B_LOC
        eS8 = [None] * B_LOC
        den_t = psd.tile([P, B_LOC, NMM], F32, name="den_t", tag="den",
                         space="PSUM")

        # p-state ramp anchor: one tiny DR matmul over a gpsimd-memset
        # scratch tile.  The ramp window is keyed to the FIRST PE matmul
        # and survives idle, so by the time the input DMAs land (~3.2us)
        # the clock is most of the way to full speed.  The garbage result
        # lands in den_t partitions/columns that every den matmul later
        # resets with start=True.
        scr = singles.tile([P, 2, 32], F8, name="scr", tag="scr")
        nc.gpsimd.memset(scr, 0)
        nc.tensor.matmul(den_t[0:32, :, :], scr[:, :, 0:32], scr[:, :, 0:16],
                         start=True, stop=True, perf_mode=DR)

        # ---- input DMAs ----
        # Every DMA's bytes flow through one shared 360B/ns engine FIFO
        # in readiness order, HWDGE issues serialize at ~650ns apiece,
        # and each completion semaphore costs +900ns.  So: the sync lane
        # carries sample-0's tensors as pieces ordered EXACTLY by first
        # consumption, and the Pool/SWDGE lane (whose descriptor
        # generation is a parallel issue path) is held back by a delay
        # memset so sample-1's bytes do not jump the FIFO ahead of the
        # critical sample-0 pieces.
        t0 = hnpool.tile([P, 2, NKO, N], F8, name="hn_s0", tag="hn")
        hn_sb.append(t0)
        m01_sb = singles.tile([P, 2, NKO, C], F8, name="m01_sb", tag="m01")
        w23_sb = singles.tile([P, 2, NKO, C], F8, name="w23_sb", tag="w23")
        nc.sync.dma_start(t0[:, 0, :, 0:NH], hn_d[0, 0, :, :, 0:NH])
        nc.sync.dma_start(m01_sb[:, 0, 0:2, :], m01_d[0, :, 0:2, :])
        nc.sync.dma_start(m01_sb[:, 0, 2:4, :], m01_d[0, :, 2:4, :])
        nc.sync.dma_start(t0[:, 0, :, NH:N], hn_d[0, 0, :, :, NH:N])
        nc.sync.dma_start(t0[:, 1, :, 0:NH], hn_d[0, 1, :, :, 0:NH])
        nc.sync.dma_start(t0[:, 1, :, NH:N], hn_d[0, 1, :, :, NH:N])
        nc.sync.dma_start(m01_sb[:, 1], m01_d[1])
        nc.sync.dma_start(w23_sb[:, 0], w23_d[0])
        nc.sync.dma_start(w23_sb[:, 1], w23_d[1])

        # Pool/SWDGE lane: a ~5.5us delay memset keeps its bytes out of
        # the early FIFO, then sample-1 hn and the small late tensors.
        delay = singles.tile([P, 6600], F8, name="delay", tag="delay")
        nc.gpsimd.memset(delay, 0)
        t1 = hnpool.tile([P, 2, NKO, N], F8, name="hn_s1", tag="hn")
        hn_sb.append(t1)
        nc.gpsimd.dma_start(t1[:, 0], hn_d[1, 0])
        nc.gpsimd.dma_start(t1[:, 1], hn_d[1, 1])
        ones_sb = singles.tile([P, 2, 1], F8, name="ones_sb", tag="ones")
        nc.gpsimd.dma_start(ones_sb, ones_d)
        r0 = singles.tile([P, NMM], F32, name="rho_s0", tag="rho0")
        nc.gpsimd.dma_start(r0, rho_d[0])
        rho_sb.append(r0)
        r1 = singles.tile([P, NMM], F32, name="rho_s1", tag="rho1")
        nc.gpsimd.dma_start(r1, rho_d[1])
        rho_sb.append(r1)

        # two-term operand pairs (hi*hi, lo*hi, hi*lo; lo*lo dropped at
        # ~0.13% magnitude).
        TERMS = ((0, 0), (1, 0), (0, 1))

        def evac2(dst_hi, dst_lo, pt):
            """PSUM -> two-term fp8: hi on ACT, lo (residual) on DVE."""
            nc.scalar.activation(dst_hi, pt, Act.Identity)
            nc.vector.tensor_tensor(dst_lo, pt, dst_hi,
                                    mybir.AluOpType.subtract)

        def front(s):
            """g = (16 M01)^T hn and vW = hn^T (16 W23): three two-term
            cross products accumulated in PSUM, evacuated to fp8 pairs."""
            hn = hn_sb[s]
            g8[s] = gpool.tile([P, 2, NKO, N], F8, name=f"g8_s{s}", tag="g8")
            vW8[s] = vpool.tile([P, 2, NMM, C], F8, name=f"vW8_s{s}",
                                tag="vW8")

            def g_tile(dc, nh):
                gt = ps.tile([P, NH], F32, name=f"g_{dc}_{nh}_s{s}",
                             tag="big", space="PSUM")
                k = 0
                for tm, th in TERMS:
                    for j in range(2):
                        nc.tensor.matmul(
                            gt, m01_sb[:, tm, 2 * j:2 * j + 2,
                                       dc * P:(dc + 1) * P],
                            hn[:, th, 2 * j:2 * j + 2, nh * NH:(nh + 1) * NH],
                            start=(k == 0), stop=(k == 5), perf_mode=DR)
                        k += 1
                evac2(g8[s][:, 0, dc, nh * NH:(nh + 1) * NH],
                      g8[s][:, 1, dc, nh * NH:(nh + 1) * NH], gt)

            def v_tile(mm):
                vt = ps.tile([P, NH], F32, name=f"v_{mm}_s{s}", tag="big",
                             space="PSUM")
                k = 0
                # hi*Whi, lo*Whi, hi*Wlo: w23-lo is the latest HWDGE
                # arrival, so it comes last.
                for th, tw in ((0, 0), (1, 0), (0, 1)):
                    for j in range(2):
                        nc.tensor.matmul(
                            vt, hn[:, th, 2 * j:2 * j + 2,
                                   mm * P:(mm + 1) * P],
                            w23_sb[:, tw, 2 * j:2 * j + 2, :],
                            start=(k == 0), stop=(k == 5), perf_mode=DR)
                        k += 1
                evac2(vW8[s][:, 0, mm, :], vW8[s][:, 1, mm, :], vt)

            if s == 0:
                # fill window: seven PSUM banks hold 4 nh0-tiles (A) and
                # 3 nh1-tiles (B); term-waves sweep them interleaved in
                # DMA arrival order -- hi*hi (hn-hi nh0 then nh1),
                # hi*lo (hn-lo), lo*hi (m01-lo, the last sync piece
                # before w23).  The leftover (3,1) tile runs solo on a
                # bank freed by the A evacuations.
                waves = ((0, 0, 0), (0, 0, 1), (0, 1, 0),
                         (0, 1, 1), (1, 0, 0), (1, 0, 1))
                tilesA = [(dc, 0) for dc in range(4)]
                tilesB = [(0, 1), (1, 1), (2, 1)]
                gts = {}
                for dc, nh in tilesA + tilesB:
                    gts[(dc, nh)] = ps.tile([P, NH], F32,
                                            name=f"g_{dc}_{nh}_s{s}",
                                            tag="big", space="PSUM")

                def g_wave(tiles, wi):
                    tm, th, j = waves[wi]
                    for dc, nh in tiles:
                        nc.tensor.matmul(
                            gts[(dc, nh)],
                            m01_sb[:, tm, 2 * j:2 * j + 2,
                                   dc * P:(dc + 1) * P],
                            hn[:, th, 2 * j:2 * j + 2,
                               nh * NH:(nh + 1) * NH],
                            start=(wi == 0), stop=(wi == 5),
                            perf_mode=DR)

                def g_evac(tiles):
                    for dc, nh in tiles:
                        evac2(g8[s][:, 0, dc, nh * NH:(nh + 1) * NH],
                              g8[s][:, 1, dc, nh * NH:(nh + 1) * NH],
                              gts[(dc, nh)])

                g_wave(tilesA, 0)
                g_wave(tilesA, 1)
                g_wave(tilesB, 0)
                g_wave(tilesB, 1)
                g_wave(tilesA, 2)
                g_wave(tilesA, 3)
                g_wave(tilesB, 2)
                g_wave(tilesB, 3)
                g_wave(tilesA, 4)
                g_wave(tilesA, 5)
                g_evac(tilesA)
                g_wave(tilesB, 4)
                g_wave(tilesB, 5)
                g_evac(tilesB)
                g_tile(3, 1)
                for u in range(8):
                    v_tile(u)
            else:
                # interleave g and vW tiles so the evacuation engines see
                # a steady stream instead of end-of-phase bursts
                for u in range(8):
                    g_tile(u // 2, u % 2)
                    v_tile(u)

        def scores(s, nh):
            """St = hn^T g8 (two-term both sides) for one n-half; exp ->
            eS8 (fp8 e5m2), key-side bias + overflow shift via rho."""
            hn = hn_sb[s]
            if eS8[s] is None:
                eS8[s] = espool.tile([P, NMM, N], F8E5, name=f"eS_s{s}",
                                     tag="eS")
            sl = slice(nh * NH, (nh + 1) * NH)
            for mm in range(NMM):
                st = ps.tile([P, NH], F32, name=f"st_{mm}_{nh}_s{s}",
                             tag="big", space="PSUM")
                k = 0
                for th, tg in TERMS:
                    for j in range(2):
                        nc.tensor.matmul(
                            st, hn[:, th, 2 * j:2 * j + 2,
                                   mm * P:(mm + 1) * P],
                            g8[s][:, tg, 2 * j:2 * j + 2, sl],
                            start=(k == 0), stop=(k == 5), perf_mode=DR)
                        k += 1
                nc.scalar.activation(eS8[s][:, mm, sl], st, Act.Exp,
                                     scale=SEXP, bias=rho_sb[s][:, mm:mm + 1])

        def tail(s, nh):
            """den columns (first, so den leaves early) + numerator
            matmuls (two-term vW) for one n-half; PSUM -> SBUF -> DMA."""
            eS = eS8[s]
            for nck in range(nh * 4, nh * 4 + 4):
                csl = slice(nck * P, (nck + 1) * P)
                for j in range(4):
                    nc.tensor.matmul(
                        den_t[:, s, nck:nck + 1],
                        eS[:, 2 * j:2 * j + 2, csl], ones_sb,
                        start=(j == 0), stop=(j == 3), perf_mode=DR)
            if nh == 1:
                dsb = singles.tile([P, NMM], F32, name=f"den_sb_s{s}",
                                   tag=f"densb{s}")
                nc.vector.tensor_copy(dsb, den_t[:, s, :])
                nc.gpsimd.dma_start(den_d[s], dsb)
            for nck in range(nh * 4, nh * 4 + 4):
                csl = slice(nck * P, (nck + 1) * P)
                if s == 1 and nck == 7:
                    # final tile: two independently-accumulated column
                    # halves, so the first half's evac+DMA chain starts
                    # while the second half's matmuls still run, and the
                    # very last chain moves only 128KB.
                    for hf in range(2):
                        nt = ps.tile([P, 256], F32, name=f"n_7{hf}_s{s}",
                                     tag="big", space="PSUM")
                        k = 0
                        for tw in range(2):
                            for j in range(4):
                                nc.tensor.matmul(
                                    nt, eS[:, 2 * j:2 * j + 2, csl],
                                    vW8[s][:, tw, 2 * j:2 * j + 2,
                                           hf * 256:(hf + 1) * 256],
                                    start=(k == 0), stop=(k == 7),
                                    perf_mode=DR)
                                k += 1
                        yh = singles.tile([P, 256], BF16, name=f"y7{hf}",
                                          tag=f"y7{hf}")
                        if hf == 0:
                            nc.vector.tensor_copy(yh, nt)
                            nc.sync.dma_start(num_d[s, nck, :, 0:256], yh)
                        else:
                            nc.scalar.activation(yh, nt, Act.Identity)
                            nc.scalar.dma_start(num_d[s, nck, :, 256:], yh)
                    continue
                nt = ps.tile([P, C], F32, name=f"n_{nck}_s{s}", tag="big",
                             space="PSUM")
                k = 0
                for tw in range(2):
                    for j in range(4):
                        nc.tensor.matmul(
                            nt, eS[:, 2 * j:2 * j + 2, csl],
                            vW8[s][:, tw, 2 * j:2 * j + 2, :],
                            start=(k == 0), stop=(k == 7), perf_mode=DR)
                        k += 1
                if s == 0 or nck in (2, 3):
                    # spread over the SWDGE lane (idle in the drain)
                    y = ypool.tile([P, C], BF16, name=f"y_{nck}_s{s}",
                                   tag="y")
                    nc.vector.tensor_copy(y, nt)
                    nc.gpsimd.dma_start(num_d[s, nck], y)
                else:
                    # sync-queue HWDGE (SP has no engine work, so its
                    # issue serialization cannot stall an evac engine);
                    # ACT takes over evacs once it is done with exp.
                    y = ypool.tile([P, C], BF16, name=f"y_{nck}_s{s}",
                                   tag="y")
                    if nh == 1 and nck % 2 == 0:
                        nc.scalar.activation(y, nt, Act.Identity)
                    else:
                        nc.vector.tensor_copy(y, nt)
                    nc.sync.dma_start(num_d[s, nck], y)

        # software pipeline: sample-1 front/scores fill PE slack while
        # sample-0's exp (ACT) and evacuations (DVE) drain, and vice versa.
        front(0)
        scores(0, 0)
        scores(0, 1)
        front(1)
        tail(0, 0)
        scores(1, 0)
        tail(0, 1)
        scores(1, 1)
        tail(1, 0)
        tail(1, 1)


_NC_CACHE = {}


def _get_nc(fast_bias=True):
    key = bool(fast_bias)
    if key not in _NC_CACHE:
        _NC_CACHE[key] = _build(key)
    return _NC_CACHE[key]


def _groupnorm_host(x, gamma, beta):
    b, c, h, w = x.shape
    xg = x.reshape(b, G, c // G, h * w)
    mu = xg.mean(axis=(2, 3), keepdims=True)
    var = xg.var(axis=(2, 3), keepdims=True)
    xn = ((xg - mu) / np.sqrt(var + EPS)).reshape(b, c, h * w)
    return xn * gamma[None, :, None] + beta[None, :, None]


def run(inputs, trace=False):
    f64 = np.float64
    W0 = np.asarray(inputs["W0"], f64)
    W1 = np.asarray(inputs["W1"], f64)
    W2 = np.asarray(inputs["W2"], f64)
    W3 = np.asarray(inputs["W3"], f64)
    b0 = np.asarray(inputs["b0"], f64)
    b2 = np.asarray(inputs["b2"], f64)
    b3 = np.asarray(inputs["b3"], f64)

    x = np.asarray(inputs["x"], np.float32)
    gamma = np.asarray(inputs["gn_gamma"], np.float32)
    beta = np.asarray(inputs["gn_beta"], np.float32)

    hn = _groupnorm_host(x, gamma, beta)              # [B, C, N] f32
    hn_hi = hn.astype(E4M3)
    hn_lo = (hn - hn_hi.astype(np.float32)).astype(E4M3)
    hn8 = np.stack([hn_hi, hn_lo], axis=1)            # [B, 2, C, N]
    # p-major: [B, 2, P, NKO, N]
    hn8 = np.ascontiguousarray(
        hn8.reshape(B_FULL, 2, NKO, P, N).transpose(0, 1, 3, 2, 4))

    M01 = (W0 @ W1.T) * WS
    W23 = (W2 @ W3) * WS
    b23 = (W3.T @ b2 + b3).astype(np.float32)
    r1 = W1 @ b0

    fast_bias = not np.any(r1)
    s = float(C) ** -0.5
    if fast_bias:
        rho = np.full((B_FULL, N), -C0, np.float32)
    else:
        # key-side bias of q.k, shifted per sample so exp() stays in the
        # fp8 range; the shift is softmax-invariant.
        rho = s * np.einsum("c,bcn->bn", r1, hn.astype(f64))
        rho = (rho - np.maximum(rho.max(axis=1, keepdims=True), 0.0)
               - C0).astype(np.float32)
    # p-major: [B, P, NMM]
    rho_pm = np.ascontiguousarray(
        rho.reshape(B_FULL, NMM, P).transpose(0, 2, 1))

    nc = _get_nc(fast_bias)

    def two_term(a):
        a = a.astype(np.float32)
        hi = a.astype(E4M3)
        lo = (a - hi.astype(np.float32)).astype(E4M3)
        pair = np.stack([hi, lo], axis=0)             # [2, C, C]
        # p-major: [2, P, NKO, C]
        return np.ascontiguousarray(
            pair.reshape(2, NKO, P, C).transpose(0, 2, 1, 3))

    base = {
        "M01": two_term(M01),
        "W23": two_term(W23),
    }
    in_maps = []
    for cid in range(NCORES):
        sl = slice(cid * B_LOC, (cid + 1) * B_LOC)
        in_maps.append(dict(base,
                            hn8=np.ascontiguousarray(hn8[sl]),
                            rho=np.ascontiguousarray(rho_pm[sl])))
    res = run_bass_kernel_spmd(nc, in_maps, list(range(NCORES)), trace=trace)

    num = np.concatenate([np.asarray(r["num"], dtype=np.float32)
                          for r in res.results], axis=0)
    den = np.concatenate([r["den"] for r in res.results], axis=0)
    # num[b, nck, p, d]: n = nck*128 + p ; den[b, p, nc]: n = nc*128 + p
    num = num.reshape(B_FULL, N, C)
    den = den.transpose(0, 2, 1).reshape(B_FULL, N)
    o = num / den[:, :, None]                          # [B, N, C]
    out = x + b23[None, :, None, None] \
        + o.transpose(0, 2, 1).reshape(B_FULL, C, H, W).astype(np.float32)
    return out, res


def kernel(**inputs) -> np.ndarray:
    out, _ = run(inputs)
    return out
